# revision 25
# baseline (speedup 1.0000x reference)
"""Trainium2 Bass kernel for nn_AttentionModel_57380763075118.

8 NeuronCores, SPMD, no collectives. core c = (batch b = c//2, half = c%2).
half0 owns frames [0,432), half1 owns [430,862) (both 432 wide -> identical
program; the 2-frame overlap is removed via a per-core column mask folded into
the iSTFT input and by host-side column slicing of the spec output).

Everything on device is feature-major [feature, time]; D is permuted to
ch-major (d = ch*1025 + bin) host-side; biases ride extra contraction rows
(ones-rows); LayerNorm statistics use ones-vector matmuls; per-feature affine
folds are precomputed host-side. Matmuls run in float32r (full PE rate, even
moving dims required -> 432/430 splits, hd padded 1025->1026).
"""
import math
import os
import sys

sys.path.insert(0, '/opt/trn_rl_repo')

import numpy as np

NFFT = 2048; HOP = 512; BINS = 1025
HEADS = 2; D = 2 * BINS; DFF = 4 * D
B = 4; L = 441000
T = 862
HD = 1025
HD_P = 1026
TQ = 432
Q_OFF = (0, T - TQ)            # (0, 430)
PAD = NFFT // 2
AUDLEN = L + 2 * PAD           # 443048 padded audio length
APLEN = NFFT + HOP * (T - 1)   # 442880 overlap-add span
SEG = (TQ - 1) * HOP + NFFT    # 222720 own segment length
NC_T = (432, 430)
KC_D = 17                      # ceil(2051/128)
KC_HD = 9
KC_T = 7
KC_N = 16
MC_DFF = 65                    # ceil(8200/128)
KC_H = 65                      # ceil(8201/128)
NC_HD = (342, 342, 342)
L2_BLK = 13                    # 5 blocks of 13 = 65
SEG_S = TQ + 3                 # 435
MC_D = 17                      # ceil(2050/128)


def _chunks(total, size=128):
    return [(i, min(size, total - i)) for i in range(0, total, size)]


# ---------------------------------------------------------------------------
# host-side constant preparation
# ---------------------------------------------------------------------------

def _perm_ch_major():
    p = np.empty(D, np.int64)
    for ch in range(2):
        p[ch * BINS:(ch + 1) * BINS] = np.arange(BINS) * 2 + ch
    return p


def _pos_enc():
    pos = np.arange(T, dtype=np.float32)[:, None]
    div = np.exp(np.arange(0, D, 2, dtype=np.float32) * (-math.log(10000.0) / D))
    pe = np.zeros((T, D), np.float32)
    pe[:, 0::2] = np.sin(pos * div)
    pe[:, 1::2] = np.cos(pos * div)
    return pe


def _prep_constants(inp):
    p = _perm_ch_major()
    window = np.asarray(inp['window'])

    n = np.arange(NFFT, dtype=np.float64)
    k = np.arange(BINS, dtype=np.float64)
    ang = 2.0 * np.pi * np.outer(n, k) / NFFT
    w64 = window.astype(np.float64)[:, None]
    stft_c = (w64 * np.cos(ang)).astype(np.float32)          # [NFFT, BINS]
    stft_s = (-w64 * np.sin(ang)).astype(np.float32)

    ang2 = ang.T                                             # [BINS, NFFT]
    coef = np.full((BINS, 1), 2.0); coef[0] = 1.0; coef[-1] = 1.0
    wrow = window.astype(np.float64)[None, :]
    istft_c = (wrow * coef * np.cos(ang2) / NFFT).astype(np.float32)
    istft_s = (-wrow * coef * np.sin(ang2) / NFFT).astype(np.float32)

    peT = np.ascontiguousarray(_pos_enc()[:, p].T)           # [D, T]

    ch_of = np.arange(D) // BINS
    def chvec(a2):
        return np.asarray(a2, np.float32)[ch_of]

    i = inp
    c1sc = np.zeros((128, 8), np.float32)
    c1wr = np.asarray(i['c1wr']); c1wi = np.asarray(i['c1wi'])
    c1br = np.asarray(i['c1br']); c1bi = np.asarray(i['c1bi'])
    for ch in range(2):
        c1sc[:, 0 + ch] = c1wr[ch]
        c1sc[:, 2 + ch] = c1wi[ch]
        c1sc[:, 4 + ch] = c1br[ch] - c1bi[ch]
        c1sc[:, 6 + ch] = c1br[ch] + c1bi[ch]

    def packd(v):
        vp = np.zeros(KC_D * 128, np.float32); vp[:D] = v
        return np.ascontiguousarray(vp.reshape(KC_D, 128).T)  # [128, KC_D]

    g2r = np.asarray(i['n2_gr'])[p]; b2r_ln = np.asarray(i['n2_br'])[p]
    g2i = np.asarray(i['n2_gi'])[p]; b2i_ln = np.asarray(i['n2_bi'])[p]
    c2wr = chvec(i['c2wr']); c2wi = chvec(i['c2wi'])
    c2br = chvec(i['c2br']); c2bi = chvec(i['c2bi'])
    mvecs = [g2r * c2wr, g2i * c2wi, b2r_ln * c2wr - b2i_ln * c2wi + c2br - c2bi,
             g2i * c2wr, g2r * c2wi, b2i_ln * c2wr + b2r_ln * c2wi + c2br + c2bi]
    dvec = np.stack([packd(v) for v in mvecs], axis=-1)      # [128, KC_D, 6]

    Win = np.asarray(i['attn_in_w']); bin_ = np.asarray(i['attn_in_b'])
    Wq, Wk, Wv = Win[:D], Win[D:2*D], Win[2*D:]
    bq, bk, bv = bin_[:D], bin_[D:2*D], bin_[2*D:]
    scale = 1.0 / np.sqrt(np.float64(HD))
    wqt, wkt, wvt = [], [], []
    for h in range(HEADS):
        for (W, b_, out, sc) in ((Wq, bq, wqt, scale), (Wk, bk, wkt, 1.0),
                                 (Wv, bv, wvt, 1.0)):
            m = np.zeros((D + 1, HD_P), np.float32)
            m[:D, :HD] = (W[h*HD:(h+1)*HD][:, p].T * sc).astype(np.float32)
            m[D, :HD] = (np.asarray(b_[h*HD:(h+1)*HD]) * sc).astype(np.float32)
            out.append(np.ascontiguousarray(m))

    Wo = np.asarray(i['attn_out_w']); bo = np.asarray(i['attn_out_b'])
    wo = np.zeros((2 * HD_P + 1, D), np.float32)
    for h in range(HEADS):
        wo[h*HD_P : h*HD_P + HD] = Wo[:, h*HD:(h+1)*HD][p, :].T
    wo[2 * HD_P] = 2.0 * bo[p]

    # FFN l1 [D+1, DFF]; LN1 gains are applied on device (per-row tensor_scalar)
    W1r = np.asarray(i['l1wr']); W1i = np.asarray(i['l1wi'])
    b1r = np.asarray(i['l1br']); b1i = np.asarray(i['l1bi'])
    g1r = np.asarray(i['n1_gr'])[p]; b1r_ln = np.asarray(i['n1_br'])[p]
    g1i = np.asarray(i['n1_gi'])[p]; b1i_ln = np.asarray(i['n1_bi'])[p]
    W1rT = W1r[:, p].T; W1iT = W1i[:, p].T                   # [D, DFF]
    w1r = np.zeros((D + 1, DFF), np.float32)
    w1i = np.zeros((D + 1, DFF), np.float32)
    w1r[:D] = W1rT
    w1i[:D] = W1iT
    # bias rows (ride ynr' ones-row): include LN1 beta folds
    w1r[D] = (b1r - b1i) + b1r_ln @ W1rT - b1i_ln @ W1iT
    w1i[D] = (b1r + b1i) + b1i_ln @ W1rT + b1r_ln @ W1iT

    W2r = np.asarray(i['l2wr']); W2i = np.asarray(i['l2wi'])
    b2r = np.asarray(i['l2br']); b2i = np.asarray(i['l2bi'])
    w2r = np.zeros((DFF + 1, D), np.float32)
    w2i = np.zeros((DFF + 1, D), np.float32)
    w2r[:DFF] = W2r[p, :].T
    w2i[:DFF] = W2i[p, :].T
    w2r[DFF] = (b2r - b2i)[p]
    w2i[DFF] = (b2r + b2i)[p]

    def pretile(w, kc_n, mc_n):
        rows = kc_n * 128; cols = mc_n * 128
        wp_ = np.zeros((rows, cols), np.float32)
        wp_[:w.shape[0], :w.shape[1]] = w
        # [kc, p, mc, m] -> [p, mc, kc, m]; contiguous (kc, m) per (p, mc)
        return np.ascontiguousarray(
            wp_.reshape(kc_n, 128, mc_n, 128).transpose(1, 2, 0, 3))

    wqp = [pretile(w, 17, 9) for w in wqt]
    wkp = [pretile(w, 17, 9) for w in wkt]
    wop = pretile(wo, 17, 17)
    stft_cp = pretile(stft_c, 16, 9)
    stft_sp = pretile(stft_s, 16, 9)
    istft_cp = pretile(istft_c, 9, 16)
    istft_sp = pretile(istft_s, 9, 16)
    w1rp = pretile(w1r, 17, 65)
    w1ip = pretile(w1i, 17, 65)
    w2rp = pretile(w2r, 65, 17)
    w2ip = pretile(w2i, 65, 17)

    return dict(w1rp=w1rp, w1ip=w1ip, w2rp=w2rp, w2ip=w2ip,
                wq0=wqp[0], wq1=wqp[1], wk0=wkp[0], wk1=wkp[1], wo=wop,
                stft_c=stft_cp, stft_s=stft_sp,
                istft_c=istft_cp, istft_s=istft_sp,
                peT=peT, c1sc=c1sc, dvec=dvec,
                g1r=packd(g1r), g1i=packd(g1i),
                wv0=wvt[0], wv1=wvt[1],
                w1r=w1r, w1i=w1i, w2r=w2r, w2i=w2i, perm=p)


def _prep_wsq(window):
    win = np.asarray(window, np.float64)
    idx = (np.arange(T)[:, None] * HOP + np.arange(NFFT)[None, :]).ravel()
    wsq = np.zeros(APLEN)
    np.add.at(wsq, idx, np.tile(win ** 2, T))
    return np.where(wsq > 1e-11, wsq, 1.0)


# ---------------------------------------------------------------------------
# bass program
# ---------------------------------------------------------------------------

def _build_program(debug=False, precise_qk=True, precise_stft=True):
    import concourse.bass as bass
    import concourse.bacc as bacc
    import concourse.tile as tile
    from concourse import mybir
    from concourse.masks import make_identity

    f32 = mybir.dt.float32
    f32r = mybir.dt.float32r
    AF = mybir.ActivationFunctionType
    ALU = mybir.AluOpType
    AX = mybir.AxisListType

    nc = bacc.Bacc("TRN2", target_bir_lowering=False, debug=False, num_devices=8)

    def din(name, shape, dt=f32r):
        return nc.dram_tensor(name, shape, dt, kind="ExternalInput")

    audio_full = din("audio_full", (2, AUDLEN), f32)
    audio_own = din("audio_own", (2, SEG), f32)
    pe_full = din("pe_full", (D, T), f32)
    pe_own = din("pe_own", (D, TQ), f32)
    c1sc_t = din("c1sc", (128, 8), f32)
    dvec_t = din("dvec", (128, KC_D, 6), f32)
    g1r_t = din("g1r", (128, KC_D), f32)
    g1i_t = din("g1i", (128, KC_D), f32)
    colmask_t = din("colmask", (1, TQ), f32)
    ones_in = din("ones_in", (1, T), f32r)
    zeros_in = din("zeros_in", (1, T), f32r)
    ones32_in = din("ones32_in", (1, T), f32)
    st_dt = f32 if precise_stft else f32r
    stft_c = din("stft_c", (128, 9, KC_N, 128), st_dt)
    stft_s = din("stft_s", (128, 9, KC_N, 128), st_dt)
    istft_c = din("istft_c", (128, KC_N, KC_HD, 128))
    istft_s = din("istft_s", (128, KC_N, KC_HD, 128))
    qk_dt = f32 if precise_qk else f32r
    wq = [din("wq0", (128, KC_HD, KC_D, 128), qk_dt),
          din("wq1", (128, KC_HD, KC_D, 128), qk_dt)]
    wk = [din("wk0", (128, KC_HD, KC_D, 128), qk_dt),
          din("wk1", (128, KC_HD, KC_D, 128), qk_dt)]
    wv = [din("wv0", (D + 1, HD_P)), din("wv1", (D + 1, HD_P))]
    wo_t = din("wo", (128, MC_D, KC_D, 128))
    w1r_t = din("w1rp", (128, MC_DFF, KC_D, 128))
    w1i_t = din("w1ip", (128, MC_DFF, KC_D, 128))
    w2r_t = din("w2rp", (128, MC_D, KC_H, 128))
    w2i_t = din("w2ip", (128, MC_D, KC_H, 128))

    spec2_r = nc.dram_tensor("spec2_r", (D, TQ), f32, kind="ExternalOutput")
    spec2_i = nc.dram_tensor("spec2_i", (D, TQ), f32, kind="ExternalOutput")
    seg_o = nc.dram_tensor("seg", (2, SEG), f32, kind="ExternalOutput")

    def dscratch(name, shape, dt=f32r):
        kind = "ExternalOutput" if debug else "Internal"
        return nc.dram_tensor(name, shape, dt, kind=kind)

    xq_d = [dscratch("xq_r", (D + 1, TQ)), dscratch("xq_i", (D + 1, TQ))]
    xt_d = [dscratch("xt_r", (D + 1, T)), dscratch("xt_i", (D + 1, T))]
    # f32 copies of x for the precise q/k path (softmax logits need fp32)
    xq32_d = [dscratch("xq32_r", (D + 1, TQ), f32),
              dscratch("xq32_i", (D + 1, TQ), f32)] if precise_qk else xq_d
    xt32_d = [dscratch("xt32_r", (D + 1, T), f32),
              dscratch("xt32_i", (D + 1, T), f32)] if precise_qk else xt_d
    specq_d = [dscratch("specq_r", (D, TQ), f32), dscratch("specq_i", (D, TQ), f32)]
    qt_d = [[dscratch(f"qt_{s}{h}", (HD_P, TQ), qk_dt) for s in "ri"]
            for h in range(2)]
    kt_d = [[dscratch(f"kt_{s}{h}", (HD_P, T), qk_dt) for s in "ri"]
            for h in range(2)]
    v_d = [[dscratch(f"v_{s}{h}", (T, HD_P)) for s in "ri"] for h in range(2)]
    ot_d = [dscratch("ot_r", (2 * HD_P + 1, TQ)),
            dscratch("ot_i", (2 * HD_P + 1, TQ))]
    h_d = [dscratch("h_r", (DFF + 1, TQ)), dscratch("h_i", (DFF + 1, TQ)),
           dscratch("h_in", (DFF + 1, TQ))]
    y_d = [dscratch("y_r", (D, TQ), f32), dscratch("y_i", (D, TQ), f32)]
    z_tap = [dscratch("z_r", (D, TQ), f32), dscratch("z_i", (D, TQ), f32)] if debug else None
    bc_d = nc.dram_tensor("bcast_scratch", (8, TQ), f32, kind="Internal")

    with tile.TileContext(nc) as tc:
      with tc.tile_pool(name="const", bufs=1) as constp:
        ident = constp.tile([128, 128], f32)
        make_identity(nc, ident[:])
        ones_col = constp.tile([128, 1], f32)
        nc.vector.memset(ones_col[:], 1.0)
        eps_t = constp.tile([1, 1], f32)
        nc.vector.memset(eps_t[:], 1e-5)
        c1sb = constp.tile([128, 8], f32)
        nc.sync.dma_start(c1sb[:], c1sc_t.ap())
        dvsb = constp.tile([128, KC_D, 6], f32)
        nc.sync.dma_start(dvsb[:], dvec_t.ap())
        g1rsb = constp.tile([128, KC_D], f32)
        nc.sync.dma_start(g1rsb[:], g1r_t.ap())
        g1isb = constp.tile([128, KC_D], f32)
        nc.sync.dma_start(g1isb[:], g1i_t.ap())
        cmask_b = constp.tile([128, TQ], f32)
        nc.sync.dma_start(cmask_b[:], bass.AP(
            tensor=colmask_t.ap().tensor, offset=0, ap=[[0, 128], [1, TQ]]))

        # ones rows for DRAM scratch (DRAM->DRAM)
        nc.sync.dma_start(xq_d[0].ap()[D:D+1, :], ones_in.ap()[:, :TQ])
        nc.sync.dma_start(xq_d[1].ap()[D:D+1, :], ones_in.ap()[:, :TQ])
        nc.sync.dma_start(xt_d[0].ap()[D:D+1, :], ones_in.ap()[:, :T])
        nc.sync.dma_start(xt_d[1].ap()[D:D+1, :], ones_in.ap()[:, :T])
        if precise_qk:
            nc.sync.dma_start(xq32_d[0].ap()[D:D+1, :], ones32_in.ap()[:, :TQ])
            nc.sync.dma_start(xq32_d[1].ap()[D:D+1, :], ones32_in.ap()[:, :TQ])
            nc.sync.dma_start(xt32_d[0].ap()[D:D+1, :], ones32_in.ap()[:, :T])
            nc.sync.dma_start(xt32_d[1].ap()[D:D+1, :], ones32_in.ap()[:, :T])
        nc.sync.dma_start(ot_d[0].ap()[2*HD_P:2*HD_P+1, :], zeros_in.ap()[:, :TQ])
        nc.sync.dma_start(ot_d[1].ap()[2*HD_P:2*HD_P+1, :], ones_in.ap()[:, :TQ])
        nc.sync.dma_start(h_d[0].ap()[DFF:DFF+1, :], ones_in.ap()[:, :TQ])
        nc.sync.dma_start(h_d[1].ap()[DFF:DFF+1, :], zeros_in.ap()[:, :TQ])
        nc.sync.dma_start(h_d[2].ap()[DFF:DFF+1, :], zeros_in.ap()[:, :TQ])

        # =================== STFT stage helper ===========================
        def stft_stage(audio_t, width, nc_chunks, pe_t, x_out, x32_out, specq_out):
            with tc.tile_pool(name="fw", bufs=1) as fwp:
                fwT = [fwp.tile([128, KC_N, width], st_dt, tag=f"fwT{c}", name=f"fwT{c}")
                       for c in range(2)]
                with tc.tile_pool(name="fr", bufs=2) as frp, \
                     tc.tile_pool(name="tr_ps", bufs=6, space="PSUM") as trps:
                    for ch in range(2):
                        for t0, tsz in _chunks(width):
                            fr_t = frp.tile([128, NFFT], f32, tag="fr")
                            src = bass.AP(
                                tensor=audio_t.ap().tensor,
                                offset=ch * audio_t.shape[1] + t0 * HOP,
                                ap=[[HOP, tsz], [1, NFFT]])
                            nc.sync.dma_start(fr_t[:tsz, :], src)
                            for nch in range(KC_N):
                                pst = trps.tile([128, 128], f32, tag="tr")
                                nc.tensor.transpose(
                                    pst[:, :tsz],
                                    fr_t[:tsz, nch*128:(nch+1)*128],
                                    ident[:tsz, :tsz])
                                nc.scalar.copy(fwT[ch][:, nch, t0:t0+tsz],
                                               pst[:, :tsz])
                with tc.tile_pool(name="st_ps", bufs=1, space="PSUM") as psp, \
                     tc.tile_pool(name="st_w", bufs=2) as wp, \
                     tc.tile_pool(name="st_t", bufs=2) as tp:
                    for mci, (mc, msz) in enumerate(_chunks(BINS)):
                        pre = [[psp.tile([128, 512], f32, tag=f"re{c}{j}", name=f"re{c}{j}")
                                for j in range(len(nc_chunks))] for c in range(2)]
                        pim = [[psp.tile([128, 512], f32, tag=f"im{c}{j}", name=f"im{c}{j}")
                                for j in range(len(nc_chunks))] for c in range(2)]
                        ct = wp.tile([128, KC_N, 128], st_dt, tag="ct")
                        nc.sync.dma_start(ct[:], stft_c.ap()[:, mci])
                        st = wp.tile([128, KC_N, 128], st_dt, tag="st")
                        nc.sync.dma_start(st[:], stft_s.ap()[:, mci])
                        for kc in range(KC_N):
                            off = 0
                            for j, nsz in enumerate(nc_chunks):
                                for c in range(2):
                                    nc.tensor.matmul(
                                        pre[c][j][:msz, :nsz], ct[:, kc, :msz],
                                        fwT[c][:, kc, off:off+nsz],
                                        start=(kc == 0), stop=(kc == KC_N - 1))
                                    nc.tensor.matmul(
                                        pim[c][j][:msz, :nsz], st[:, kc, :msz],
                                        fwT[c][:, kc, off:off+nsz],
                                        start=(kc == 0), stop=(kc == KC_N - 1))
                                off += nsz
                        for c in range(2):
                            row = c * BINS + mc
                            off = 0
                            for j, nsz in enumerate(nc_chunks):
                                ps_re = pre[c][j][:msz, :nsz]
                                ps_im = pim[c][j][:msz, :nsz]
                                if specq_out is not None:
                                    sq = tp.tile([128, 512], f32, tag="sq")
                                    nc.scalar.copy(sq[:msz, :nsz], ps_re)
                                    nc.sync.dma_start(
                                        specq_out[0].ap()[row:row+msz, off:off+nsz],
                                        sq[:msz, :nsz])
                                    sq2 = tp.tile([128, 512], f32, tag="sq2")
                                    nc.scalar.copy(sq2[:msz, :nsz], ps_im)
                                    nc.sync.dma_start(
                                        specq_out[1].ap()[row:row+msz, off:off+nsz],
                                        sq2[:msz, :nsz])
                                pe_sb = tp.tile([128, 512], f32, tag="pe")
                                nc.sync.dma_start(
                                    pe_sb[:msz, :nsz],
                                    pe_t.ap()[row:row+msz, off:off+nsz])
                                t1 = tp.tile([128, 512], f32, tag="t1")
                                nc.vector.tensor_scalar(
                                    out=t1[:msz, :nsz], in0=ps_re,
                                    scalar1=c1sb[:msz, 0+c:1+c],
                                    scalar2=c1sb[:msz, 4+c:5+c],
                                    op0=ALU.mult, op1=ALU.add)
                                t2 = tp.tile([128, 512], f32, tag="t2")
                                nc.vector.tensor_scalar(
                                    out=t2[:msz, :nsz], in0=ps_im,
                                    scalar1=c1sb[:msz, 2+c:3+c], scalar2=None,
                                    op0=ALU.mult)
                                nc.vector.tensor_sub(t1[:msz, :nsz],
                                                     t1[:msz, :nsz], t2[:msz, :nsz])
                                xo32 = tp.tile([128, 512], f32, tag="xo32")
                                nc.vector.tensor_add(xo32[:msz, :nsz],
                                                     t1[:msz, :nsz], pe_sb[:msz, :nsz])
                                xo = tp.tile([128, 512], f32r, tag="xo")
                                nc.scalar.copy(xo[:msz, :nsz], xo32[:msz, :nsz])
                                nc.sync.dma_start(
                                    x_out[0].ap()[row:row+msz, off:off+nsz],
                                    xo[:msz, :nsz])
                                if precise_qk:
                                    nc.sync.dma_start(
                                        x32_out[0].ap()[row:row+msz, off:off+nsz],
                                        xo32[:msz, :nsz])
                                t3 = tp.tile([128, 512], f32, tag="t3")
                                nc.vector.tensor_scalar(
                                    out=t3[:msz, :nsz], in0=ps_im,
                                    scalar1=c1sb[:msz, 0+c:1+c],
                                    scalar2=c1sb[:msz, 6+c:7+c],
                                    op0=ALU.mult, op1=ALU.add)
                                t4 = tp.tile([128, 512], f32, tag="t4")
                                nc.vector.tensor_scalar(
                                    out=t4[:msz, :nsz], in0=ps_re,
                                    scalar1=c1sb[:msz, 2+c:3+c], scalar2=None,
                                    op0=ALU.mult)
                                nc.vector.tensor_add(t3[:msz, :nsz],
                                                     t3[:msz, :nsz], t4[:msz, :nsz])
                                xo232 = tp.tile([128, 512], f32, tag="xo232")
                                nc.vector.tensor_add(xo232[:msz, :nsz],
                                                     t3[:msz, :nsz], pe_sb[:msz, :nsz])
                                xo2 = tp.tile([128, 512], f32r, tag="xo2")
                                nc.scalar.copy(xo2[:msz, :nsz], xo232[:msz, :nsz])
                                nc.sync.dma_start(
                                    x_out[1].ap()[row:row+msz, off:off+nsz],
                                    xo2[:msz, :nsz])
                                if precise_qk:
                                    nc.sync.dma_start(
                                        x32_out[1].ap()[row:row+msz, off:off+nsz],
                                        xo232[:msz, :nsz])
                                off += nsz

        # =================== stage A: own STFT ===========================
        stft_stage(audio_own, TQ, (TQ,), pe_own, xq_d, xq32_d, specq_d)

        # =================== in_proj Q ===================================
        with tc.tile_pool(name="xq", bufs=1) as xqp:
            xq_sb = []
            for s in range(2):
                xt_ = xqp.tile([128, KC_D, TQ], qk_dt, tag=f"xq{s}")
                for kc, (r0, rsz) in enumerate(_chunks(D + 1)):
                    nc.sync.dma_start(xt_[:rsz, kc, :], xq32_d[s].ap()[r0:r0+rsz, :])
                xq_sb.append(xt_)
            with tc.tile_pool(name="q_ps", bufs=2, space="PSUM") as psp, \
                 tc.tile_pool(name="q_w", bufs=4) as wp, \
                 tc.tile_pool(name="q_o", bufs=4) as op_:
                for h in range(2):
                    for mci, (mc, msz) in enumerate(_chunks(HD_P)):
                        pq = [psp.tile([128, TQ], f32, tag=f"pq{s}", name=f"pq{s}")
                              for s in range(2)]
                        wt = wp.tile([128, KC_D, 128], qk_dt, tag="wt")
                        nc.sync.dma_start(wt[:], wq[h].ap()[:, mci])
                        for kc, (r0, rsz) in enumerate(_chunks(D + 1)):
                            for s in range(2):
                                nc.tensor.matmul(pq[s][:msz, :],
                                                 wt[:rsz, kc, :msz],
                                                 xq_sb[s][:rsz, kc, :],
                                                 start=(kc == 0),
                                                 stop=(kc == KC_D - 1))
                        for s in range(2):
                            ot = op_.tile([128, TQ], qk_dt, tag="qo")
                            nc.scalar.copy(ot[:msz, :], pq[s][:msz, :])
                            nc.sync.dma_start(qt_d[h][s].ap()[mc:mc+msz, :],
                                              ot[:msz, :])

        # =================== stage B: full STFT ==========================
        stft_stage(audio_full, T, NC_T, pe_full, xt_d, xt32_d, None)

        # =================== in_proj K and V =============================
        with tc.tile_pool(name="xtk", bufs=1) as xtkp:
            xt32_sb = []     # qk-dtype x (for K)
            for s in range(2):
                xt3 = xtkp.tile([128, KC_D, T], qk_dt, tag=f"xt32{s}",
                                name=f"xt32{s}")
                for kc, (r0, rsz) in enumerate(_chunks(D + 1)):
                    nc.sync.dma_start(xt3[:rsz, kc, :],
                                      xt32_d[s].ap()[r0:r0+rsz, :])
                xt32_sb.append(xt3)
            with tc.tile_pool(name="k_ps", bufs=2, space="PSUM") as psp, \
                 tc.tile_pool(name="k_w", bufs=4) as wp, \
                 tc.tile_pool(name="k_o", bufs=4) as op_:
                for h in range(2):
                    for mci, (mc, msz) in enumerate(_chunks(HD_P)):
                        pk = [[psp.tile([128, 512], f32, tag=f"pk{s}{j}", name=f"pk{s}{j}")
                               for j in range(2)] for s in range(2)]
                        wt = wp.tile([128, KC_D, 128], qk_dt, tag="wt")
                        nc.sync.dma_start(wt[:], wk[h].ap()[:, mci])
                        for kc, (r0, rsz) in enumerate(_chunks(D + 1)):
                            off = 0
                            for j, nsz in enumerate(NC_T):
                                for s in range(2):
                                    nc.tensor.matmul(
                                        pk[s][j][:msz, :nsz], wt[:rsz, kc, :msz],
                                        xt32_sb[s][:rsz, kc, off:off+nsz],
                                        start=(kc == 0), stop=(kc == KC_D - 1))
                                off += nsz
                        off = 0
                        for j, nsz in enumerate(NC_T):
                            for s in range(2):
                                ot = op_.tile([128, 512], qk_dt, tag="ko")
                                nc.scalar.copy(ot[:msz, :nsz], pk[s][j][:msz, :nsz])
                                nc.sync.dma_start(
                                    kt_d[h][s].ap()[mc:mc+msz, off:off+nsz],
                                    ot[:msz, :nsz])
                            off += nsz
        with tc.tile_pool(name="xtv", bufs=1) as xtvp:
            xt_sb = []
            for s in range(2):
                xt_ = xtvp.tile([128, KC_D, T], f32r, tag=f"xt{s}")
                for kc, (r0, rsz) in enumerate(_chunks(D + 1)):
                    nc.sync.dma_start(xt_[:rsz, kc, :], xt_d[s].ap()[r0:r0+rsz, :])
                xt_sb.append(xt_)
            with tc.tile_pool(name="v_ps", bufs=3, space="PSUM") as psp, \
                 tc.tile_pool(name="v_w", bufs=1) as wp, \
                 tc.tile_pool(name="v_o", bufs=4) as op_:
                for h in range(2):
                    woff = 0
                    for nsz in NC_HD:
                        wv_sb = wp.tile([128, KC_D, nsz], f32r, tag="wv_sb")
                        for kc, (r0, rsz) in enumerate(_chunks(D + 1)):
                            nc.sync.dma_start(
                                wv_sb[:rsz, kc, :],
                                wv[h].ap()[r0:r0+rsz, woff:woff+nsz])
                        for s in range(2):
                            for tc_, tsz in _chunks(T):
                                pv = psp.tile([128, 512], f32, tag="pv")
                                for kc, (r0, rsz) in enumerate(_chunks(D + 1)):
                                    nc.tensor.matmul(
                                        pv[:tsz, :nsz],
                                        xt_sb[s][:rsz, kc, tc_:tc_+tsz],
                                        wv_sb[:rsz, kc, :],
                                        start=(kc == 0), stop=(kc == KC_D - 1))
                                vo = op_.tile([128, 512], f32r, tag="vo")
                                nc.scalar.copy(vo[:tsz, :nsz], pv[:tsz, :nsz])
                                nc.sync.dma_start(
                                    v_d[h][s].ap()[tc_:tc_+tsz, woff:woff+nsz],
                                    vo[:tsz, :nsz])
                        woff += nsz

        # =================== attention per head ==========================
        for h in range(2):
            with tc.tile_pool(name="at_pmn", bufs=1) as mp_:
                pmn_sb = mp_.tile([128, 3, KC_T, TQ], f32r)
                with tc.tile_pool(name="at_qk", bufs=1) as qkp, \
                     tc.tile_pool(name="at_ps", bufs=2, space="PSUM") as psp, \
                     tc.tile_pool(name="tr_ps2", bufs=4, space="PSUM") as trps, \
                     tc.tile_pool(name="at_e", bufs=1) as ep, \
                     tc.tile_pool(name="at_t", bufs=2) as tp:
                    qt_sb, kt_sb = [], []
                    for s in range(2):
                        q_ = qkp.tile([128, KC_HD, TQ], qk_dt, tag=f"q{s}")
                        for kc, (r0, rsz) in enumerate(_chunks(HD_P)):
                            nc.sync.dma_start(q_[:rsz, kc, :],
                                              qt_d[h][s].ap()[r0:r0+rsz, :])
                        qt_sb.append(q_)
                        k_ = qkp.tile([128, KC_HD, T], qk_dt, tag=f"k{s}")
                        for kc, (r0, rsz) in enumerate(_chunks(HD_P)):
                            nc.sync.dma_start(k_[:rsz, kc, :],
                                              kt_d[h][s].ap()[r0:r0+rsz, :])
                        kt_sb.append(k_)

                    def softmax_map(a, b_, etag):
                        e_t = ep.tile([128, 4, T], f32, tag=etag)
                        rden = tp.tile([128, 4, 1], f32, tag=f"rd{etag}")
                        for mqi, (q0, qsz) in enumerate(_chunks(TQ)):
                            ps = [psp.tile([128, 512], f32, tag=f"s{j}", name=f"sps{j}")
                                  for j in range(2)]
                            for kc, (r0, rsz) in enumerate(_chunks(HD_P)):
                                off = 0
                                for j, nsz in enumerate(NC_T):
                                    nc.tensor.matmul(
                                        ps[j][:qsz, :nsz],
                                        qt_sb[a][:rsz, kc, q0:q0+qsz],
                                        kt_sb[b_][:rsz, kc, off:off+nsz],
                                        start=(kc == 0), stop=(kc == KC_HD - 1))
                                    off += nsz
                            mx = tp.tile([128, 2], f32, tag="mx")
                            nc.vector.reduce_max(mx[:qsz, 0:1],
                                                 ps[0][:qsz, :NC_T[0]], axis=AX.X)
                            nc.vector.reduce_max(mx[:qsz, 1:2],
                                                 ps[1][:qsz, :NC_T[1]], axis=AX.X)
                            nmx = tp.tile([128, 1], f32, tag="nmx")
                            nc.vector.reduce_max(nmx[:qsz, :], mx[:qsz, :], axis=AX.X)
                            nc.vector.tensor_scalar_mul(nmx[:qsz, :],
                                                        nmx[:qsz, :], -1.0)
                            dn = tp.tile([128, 2], f32, tag="dn")
                            off = 0
                            for j, nsz in enumerate(NC_T):
                                nc.scalar.activation(
                                    out=e_t[:qsz, mqi, off:off+nsz],
                                    in_=ps[j][:qsz, :nsz], func=AF.Exp,
                                    bias=nmx[:qsz, :], scale=1.0,
                                    accum_out=dn[:qsz, j:j+1])
                                off += nsz
                            nc.vector.tensor_add(dn[:qsz, 0:1], dn[:qsz, 0:1],
                                                 dn[:qsz, 1:2])
                            nc.vector.reciprocal(rden[:qsz, mqi, :], dn[:qsz, 0:1])
                        return e_t, rden

                    def combine_transpose(ea, rda, eb, rdb, dst_idx, sub):
                        for mqi, (q0, qsz) in enumerate(_chunks(TQ)):
                            t1 = tp.tile([128, T], f32, tag="c1")
                            nc.vector.tensor_scalar(
                                out=t1[:qsz, :], in0=ea[:qsz, mqi, :],
                                scalar1=rda[:qsz, mqi, :], scalar2=None,
                                op0=ALU.mult)
                            t2 = tp.tile([128, T], f32, tag="c2")
                            nc.vector.tensor_scalar(
                                out=t2[:qsz, :], in0=eb[:qsz, mqi, :],
                                scalar1=rdb[:qsz, mqi, :], scalar2=None,
                                op0=ALU.mult)
                            cm = tp.tile([128, T], f32, tag="cm")
                            if sub:
                                nc.vector.tensor_sub(cm[:qsz, :], t1[:qsz, :],
                                                     t2[:qsz, :])
                            else:
                                nc.vector.tensor_add(cm[:qsz, :], t1[:qsz, :],
                                                     t2[:qsz, :])
                            for kci, (k0, ksz) in enumerate(_chunks(T)):
                                pst = trps.tile([128, 128], f32, tag="trm")
                                nc.tensor.transpose(pst[:ksz, :qsz],
                                                    cm[:qsz, k0:k0+ksz],
                                                    ident[:qsz, :qsz])
                                nc.scalar.copy(
                                    pmn_sb[:ksz, dst_idx, kci, q0:q0+qsz],
                                    pst[:ksz, :qsz])

                    e_rr, rd_rr = softmax_map(0, 0, "eA")
                    e_ii, rd_ii = softmax_map(1, 1, "eB")
                    combine_transpose(e_rr, rd_rr, e_ii, rd_ii, 0, True)   # P
                    e_ri, rd_ri = softmax_map(0, 1, "eA")
                    e_ir, rd_ir = softmax_map(1, 0, "eB")
                    combine_transpose(e_ri, rd_ri, e_ir, rd_ir, 1, True)   # M
                    combine_transpose(e_ri, rd_ri, e_ir, rd_ir, 2, False)  # N

                # applies: oT_r = v_r.T@P + v_i.T@M ; oT_i = v_i.T@P + v_r.T@N
                with tc.tile_pool(name="at_v", bufs=1) as vp, \
                     tc.tile_pool(name="ap_ps", bufs=4, space="PSUM") as psp, \
                     tc.tile_pool(name="ap_o", bufs=4) as op_:
                    v_sb = []
                    for s in range(2):
                        v_ = vp.tile([128, KC_T, HD_P], f32r, tag=f"v{s}")
                        for kc, (r0, rsz) in enumerate(_chunks(T)):
                            nc.sync.dma_start(v_[:rsz, kc, :],
                                              v_d[h][s].ap()[r0:r0+rsz, :])
                        v_sb.append(v_)
                    for si, (va, ma, vb, mb) in enumerate(
                            ((0, 0, 1, 1), (1, 0, 0, 2))):
                        for mc, msz in _chunks(HD_P):
                            pa = psp.tile([128, TQ], f32, tag="pa")
                            for kc, (r0, rsz) in enumerate(_chunks(T)):
                                nc.tensor.matmul(
                                    pa[:msz, :], v_sb[va][:rsz, kc, mc:mc+msz],
                                    pmn_sb[:rsz, ma, kc, :],
                                    start=(kc == 0), stop=False)
                            for kc, (r0, rsz) in enumerate(_chunks(T)):
                                nc.tensor.matmul(
                                    pa[:msz, :], v_sb[vb][:rsz, kc, mc:mc+msz],
                                    pmn_sb[:rsz, mb, kc, :],
                                    start=False, stop=(kc == KC_T - 1))
                            oo = op_.tile([128, TQ], f32r, tag="oo")
                            nc.scalar.copy(oo[:msz, :], pa[:msz, :])
                            nc.sync.dma_start(
                                ot_d[si].ap()[h*HD_P+mc : h*HD_P+mc+msz, :],
                                oo[:msz, :])

        # =================== out_proj (+ fused LN1 stats) ================
        with tc.tile_pool(name="op_ot", bufs=1) as otp, \
             tc.tile_pool(name="op_ps", bufs=2, space="PSUM") as psp, \
             tc.tile_pool(name="st_ps2", bufs=1, space="PSUM") as stp, \
             tc.tile_pool(name="op_w", bufs=2) as wp, \
             tc.tile_pool(name="op_t", bufs=4) as otmp:
            ot_sb = []
            for s in range(2):
                o_ = otp.tile([128, KC_D, TQ], f32r, tag=f"ot{s}", name=f"ot{s}")
                for kc, (r0, rsz) in enumerate(_chunks(2 * HD_P + 1)):
                    nc.sync.dma_start(o_[:rsz, kc, :], ot_d[s].ap()[r0:r0+rsz, :])
                ot_sb.append(o_)
            pm = [stp.tile([1, TQ], f32, tag=f"pm{s}", name=f"pm{s}")
                  for s in range(2)]
            pv = [stp.tile([1, TQ], f32, tag=f"pv{s}", name=f"pv{s}")
                  for s in range(2)]
            for mci, (mc, msz) in enumerate(_chunks(D)):
                py = [psp.tile([128, TQ], f32, tag=f"py{s}", name=f"py{s}")
                      for s in range(2)]
                wt = wp.tile([128, KC_D, 128], f32r, tag="wt")
                nc.sync.dma_start(wt[:], wo_t.ap()[:, mci])
                for kc, (r0, rsz) in enumerate(_chunks(2 * HD_P + 1)):
                    for s in range(2):
                        nc.tensor.matmul(py[s][:msz, :], wt[:rsz, kc, :msz],
                                         ot_sb[s][:rsz, kc, :],
                                         start=(kc == 0), stop=(kc == KC_D - 1))
                for s in range(2):
                    yc = otmp.tile([128, TQ], f32, tag=f"yc{s}")
                    nc.scalar.copy(yc[:msz, :], py[s][:msz, :])
                    nc.sync.dma_start(y_d[s].ap()[mc:mc+msz, :], yc[:msz, :])
                    nc.tensor.matmul(pm[s][:1, :], ones_col[:msz, :], yc[:msz, :],
                                     start=(mci == 0), stop=(mci == MC_D_LAST))
                    sq = otmp.tile([128, TQ], f32, tag=f"sq{s}")
                    nc.scalar.activation(out=sq[:msz, :], in_=yc[:msz, :],
                                         func=AF.Square)
                    nc.tensor.matmul(pv[s][:1, :], ones_col[:msz, :], sq[:msz, :],
                                     start=(mci == 0), stop=(mci == MC_D_LAST))
            # finalize LN1 stats -> DRAM-bounce broadcast tiles kept in a
            # small long-lived pool for the normalize pass below
            with tc.tile_pool(name="ln_bc", bufs=1) as bcp:
                stats = []
                for s in range(2):
                    mT = otmp.tile([1, TQ], f32, tag=f"m{s}")
                    nc.scalar.mul(mT[:1, :], pm[s][:1, :], 1.0 / D)
                    msq = otmp.tile([1, TQ], f32, tag=f"msq{s}")
                    nc.scalar.activation(out=msq[:1, :], in_=mT[:1, :],
                                         func=AF.Square)
                    var = otmp.tile([1, TQ], f32, tag=f"var{s}")
                    nc.scalar.mul(var[:1, :], pv[s][:1, :], 1.0 / D)
                    nc.vector.tensor_sub(var[:1, :], var[:1, :], msq[:1, :])
                    nc.scalar.activation(out=var[:1, :], in_=var[:1, :],
                                         func=AF.Sqrt, bias=eps_t[:1, :])
                    rs = otmp.tile([1, TQ], f32, tag=f"rs{s}")
                    nc.vector.reciprocal(rs[:1, :], var[:1, :])
                    nc.sync.dma_start(bc_d.ap()[2*s:2*s+1, :], mT[:1, :])
                    nc.sync.dma_start(bc_d.ap()[2*s+1:2*s+2, :], rs[:1, :])
                    mb = bcp.tile([128, TQ], f32, tag=f"mb{s}", name=f"mb{s}")
                    nc.sync.dma_start(mb[:], bass.AP(
                        tensor=bc_d.ap().tensor, offset=2*s*TQ,
                        ap=[[0, 128], [1, TQ]]))
                    rb = bcp.tile([128, TQ], f32, tag=f"rb{s}", name=f"rb{s}")
                    nc.sync.dma_start(rb[:], bass.AP(
                        tensor=bc_d.ap().tensor, offset=(2*s+1)*TQ,
                        ap=[[0, 128], [1, TQ]]))
                    stats.append((mb, rb))

        # =================== LN1 normalize + FFN l1 ======================
        with tc.tile_pool(name="ynp", bufs=1) as ynp:
            yn_sb = [ynp.tile([128, KC_D, TQ], f32r, tag=f"yn{v}", name=f"yn{v}")
                     for v in range(3)]           # ynr', yni'', neg-yni''
            with tc.tile_pool(name="ln_t", bufs=3) as tp:
                for kc, (r0, rsz) in enumerate(_chunks(D)):
                    for s in range(2):
                        mb, rb = stats[s]
                        yl = tp.tile([128, TQ], f32, tag=f"yl{s}")
                        nc.sync.dma_start(yl[:rsz, :], y_d[s].ap()[r0:r0+rsz, :])
                        t_ = tp.tile([128, TQ], f32, tag=f"n{s}")
                        nc.vector.tensor_sub(t_[:rsz, :], yl[:rsz, :],
                                             mb[:rsz, :])
                        nc.vector.tensor_mul(t_[:rsz, :], t_[:rsz, :],
                                             rb[:rsz, :])
                        if s == 0:
                            nc.vector.tensor_scalar(
                                out=yn_sb[0][:rsz, kc, :], in0=t_[:rsz, :],
                                scalar1=g1rsb[:rsz, kc:kc+1], scalar2=None,
                                op0=ALU.mult)
                        else:
                            nc.vector.tensor_scalar(
                                out=yn_sb[1][:rsz, kc, :], in0=t_[:rsz, :],
                                scalar1=g1isb[:rsz, kc:kc+1], scalar2=None,
                                op0=ALU.mult)
                            nc.vector.tensor_scalar(
                                out=yn_sb[2][:rsz, kc, :], in0=t_[:rsz, :],
                                scalar1=g1isb[:rsz, kc:kc+1], scalar2=-1.0,
                                op0=ALU.mult, op1=ALU.mult)
                lastc = (D + 1 - 1) // 128
                lastp = D - lastc * 128
                nc.sync.dma_start(yn_sb[0][lastp:lastp+1, lastc, :],
                                  ones_in.ap()[:, :TQ])
                nc.sync.dma_start(yn_sb[1][lastp:lastp+1, lastc, :],
                                  zeros_in.ap()[:, :TQ])
                nc.sync.dma_start(yn_sb[2][lastp:lastp+1, lastc, :],
                                  zeros_in.ap()[:, :TQ])

            # FFN l1 with batched (pre-tiled) weight loads
            with tc.tile_pool(name="l1_ps", bufs=4, space="PSUM") as psp, \
                 tc.tile_pool(name="l1_w", bufs=2) as wp, \
                 tc.tile_pool(name="l1_o", bufs=3) as op_:
                for mci, (mc, msz) in enumerate(_chunks(DFF)):
                    wtr = wp.tile([128, KC_D, 128], f32r, tag="wtr")
                    nc.sync.dma_start(wtr[:], w1r_t.ap()[:, mci])
                    wti = wp.tile([128, KC_D, 128], f32r, tag="wti")
                    nc.sync.dma_start(wti[:], w1i_t.ap()[:, mci])
                    phr = psp.tile([128, TQ], f32, tag="phr")
                    phi = psp.tile([128, TQ], f32, tag="phi")
                    for kc, (r0, rsz) in enumerate(_chunks(D + 1)):
                        nc.tensor.matmul(phr[:msz, :], wtr[:rsz, kc, :msz],
                                         yn_sb[0][:rsz, kc, :],
                                         start=(kc == 0), stop=False)
                        nc.tensor.matmul(phr[:msz, :], wti[:rsz, kc, :msz],
                                         yn_sb[2][:rsz, kc, :],
                                         start=False, stop=(kc == KC_D - 1))
                        nc.tensor.matmul(phi[:msz, :], wti[:rsz, kc, :msz],
                                         yn_sb[0][:rsz, kc, :],
                                         start=(kc == 0), stop=False)
                        nc.tensor.matmul(phi[:msz, :], wtr[:rsz, kc, :msz],
                                         yn_sb[1][:rsz, kc, :],
                                         start=False, stop=(kc == KC_D - 1))
                    hro = op_.tile([128, TQ], f32r, tag="hro")
                    nc.scalar.activation(out=hro[:msz, :], in_=phr[:msz, :],
                                         func=AF.Relu)
                    nc.sync.dma_start(h_d[0].ap()[mc:mc+msz, :], hro[:msz, :])
                    hio = op_.tile([128, TQ], f32r, tag="hio")
                    nc.scalar.activation(out=hio[:msz, :], in_=phi[:msz, :],
                                         func=AF.Relu)
                    nc.sync.dma_start(h_d[1].ap()[mc:mc+msz, :], hio[:msz, :])
                    hin = op_.tile([128, TQ], f32r, tag="hin")
                    nc.vector.tensor_scalar(
                        out=hin[:msz, :], in0=phi[:msz, :], scalar1=-1.0,
                        scalar2=0.0, op0=ALU.mult, op1=ALU.min)
                    nc.sync.dma_start(h_d[2].ap()[mc:mc+msz, :], hin[:msz, :])

        # =================== FFN l2 (kc-blocked, zT accum in SBUF) =======
        with tc.tile_pool(name="l2_acc", bufs=1) as accp:
            z_sb = [accp.tile([128, KC_D, TQ], f32, tag=f"z{s}", name=f"zacc{s}") for s in range(2)]
            with tc.tile_pool(name="l2_h", bufs=1) as hp, \
                 tc.tile_pool(name="l2_ps", bufs=2, space="PSUM") as psp, \
                 tc.tile_pool(name="l2_w", bufs=6) as wp:
                nblk = (KC_H + L2_BLK - 1) // L2_BLK
                for blk in range(nblk):
                    k0 = blk * L2_BLK
                    kn = min(L2_BLK, KC_H - k0)
                    hb = [hp.tile([128, L2_BLK, TQ], f32r, tag=f"hb{v}", name=f"hb{v}")
                          for v in range(3)]
                    for v in range(3):
                        for kk in range(kn):
                            r0 = (k0 + kk) * 128
                            rsz = min(128, DFF + 1 - r0)
                            nc.sync.dma_start(hb[v][:rsz, kk, :],
                                              h_d[v].ap()[r0:r0+rsz, :])
                    for mc, msz in _chunks(D):
                        mci = mc // 128
                        pzr = psp.tile([128, TQ], f32, tag="pzr")
                        pzi = psp.tile([128, TQ], f32, tag="pzi")
                        mci = mc // 128
                        wtr = wp.tile([128, L2_BLK, 128], f32r, tag="wtr")
                        nc.sync.dma_start(wtr[:, :kn, :],
                                          w2r_t.ap()[:, mci, k0:k0+kn, :])
                        wti = wp.tile([128, L2_BLK, 128], f32r, tag="wti")
                        nc.sync.dma_start(wti[:, :kn, :],
                                          w2i_t.ap()[:, mci, k0:k0+kn, :])
                        for kk in range(kn):
                            r0 = (k0 + kk) * 128
                            rsz = min(128, DFF + 1 - r0)
                            nc.tensor.matmul(pzr[:msz, :], wtr[:rsz, kk, :msz],
                                             hb[0][:rsz, kk, :],
                                             start=(kk == 0), stop=False)
                            nc.tensor.matmul(pzr[:msz, :], wti[:rsz, kk, :msz],
                                             hb[2][:rsz, kk, :],
                                             start=False, stop=(kk == kn - 1))
                            nc.tensor.matmul(pzi[:msz, :], wti[:rsz, kk, :msz],
                                             hb[0][:rsz, kk, :],
                                             start=(kk == 0), stop=False)
                            nc.tensor.matmul(pzi[:msz, :], wtr[:rsz, kk, :msz],
                                             hb[1][:rsz, kk, :],
                                             start=False, stop=(kk == kn - 1))
                        if blk == 0:
                            nc.scalar.copy(z_sb[0][:msz, mci, :], pzr[:msz, :])
                            nc.scalar.copy(z_sb[1][:msz, mci, :], pzi[:msz, :])
                        else:
                            nc.vector.tensor_add(z_sb[0][:msz, mci, :],
                                                 z_sb[0][:msz, mci, :],
                                                 pzr[:msz, :])
                            nc.vector.tensor_add(z_sb[1][:msz, mci, :],
                                                 z_sb[1][:msz, mci, :],
                                                 pzi[:msz, :])
            if debug:
                for s in range(2):
                    for kc, (r0, rsz) in enumerate(_chunks(D)):
                        nc.sync.dma_start(z_tap[s].ap()[r0:r0+rsz, :],
                                          z_sb[s][:rsz, kc, :])

            # =================== LN2 + mask + spec2 ======================
            sp2m = [nc.dram_tensor(f"sp2m_{s}", (D, TQ), f32r, kind="Internal")
                    for s in range(2)]
            with tc.tile_pool(name="ln2_ps", bufs=1, space="PSUM") as psp, \
                 tc.tile_pool(name="ln2_t", bufs=2) as tp:
                stats2 = []
                for s in range(2):
                    pm = psp.tile([1, TQ], f32, tag=f"pm{s}")
                    pv = psp.tile([1, TQ], f32, tag=f"pv{s}")
                    for kc, (r0, rsz) in enumerate(_chunks(D)):
                        nc.tensor.matmul(pm[:1, :], ones_col[:rsz, :],
                                         z_sb[s][:rsz, kc, :],
                                         start=(kc == 0), stop=(kc == MC_D_LAST))
                        sq = tp.tile([128, TQ], f32, tag="sq")
                        nc.scalar.activation(out=sq[:rsz, :],
                                             in_=z_sb[s][:rsz, kc, :], func=AF.Square)
                        nc.tensor.matmul(pv[:1, :], ones_col[:rsz, :], sq[:rsz, :],
                                         start=(kc == 0), stop=(kc == MC_D_LAST))
                    mT = tp.tile([1, TQ], f32, tag=f"m{s}")
                    nc.scalar.mul(mT[:1, :], pm[:1, :], 1.0 / D)
                    msq = tp.tile([1, TQ], f32, tag=f"msq{s}")
                    nc.scalar.activation(out=msq[:1, :], in_=mT[:1, :], func=AF.Square)
                    var = tp.tile([1, TQ], f32, tag=f"var{s}")
                    nc.scalar.mul(var[:1, :], pv[:1, :], 1.0 / D)
                    nc.vector.tensor_sub(var[:1, :], var[:1, :], msq[:1, :])
                    nc.scalar.activation(out=var[:1, :], in_=var[:1, :],
                                         func=AF.Sqrt, bias=eps_t[:1, :])
                    rs = tp.tile([1, TQ], f32, tag=f"rs{s}")
                    nc.vector.reciprocal(rs[:1, :], var[:1, :])
                    nc.sync.dma_start(bc_d.ap()[4+2*s:5+2*s, :], mT[:1, :])
                    nc.sync.dma_start(bc_d.ap()[5+2*s:6+2*s, :], rs[:1, :])
                    mb = tp.tile([128, TQ], f32, tag=f"mb{s}")
                    nc.sync.dma_start(mb[:], bass.AP(
                        tensor=bc_d.ap().tensor, offset=(4+2*s)*TQ,
                        ap=[[0, 128], [1, TQ]]))
                    rb = tp.tile([128, TQ], f32, tag=f"rb{s}")
                    nc.sync.dma_start(rb[:], bass.AP(
                        tensor=bc_d.ap().tensor, offset=(5+2*s)*TQ,
                        ap=[[0, 128], [1, TQ]]))
                    stats2.append((mb, rb))
                for kc, (r0, rsz) in enumerate(_chunks(D)):
                    zn = []
                    for s in range(2):
                        mb, rb = stats2[s]
                        t_ = tp.tile([128, TQ], f32, tag=f"zn{s}")
                        nc.vector.tensor_sub(t_[:rsz, :], z_sb[s][:rsz, kc, :],
                                             mb[:rsz, :])
                        nc.vector.tensor_mul(t_[:rsz, :], t_[:rsz, :], rb[:rsz, :])
                        zn.append(t_)
                    sqr = tp.tile([128, TQ], f32, tag="sqr")
                    nc.sync.dma_start(sqr[:rsz, :], specq_d[0].ap()[r0:r0+rsz, :])
                    sqi = tp.tile([128, TQ], f32, tag="sqi")
                    nc.sync.dma_start(sqi[:rsz, :], specq_d[1].ap()[r0:r0+rsz, :])
                    # m_r = zn_r*Ar - zn_i*Br + Cr ; m_i = zn_i*Ai + zn_r*Bi + Ci
                    t1 = tp.tile([128, TQ], f32, tag="mk1")
                    nc.vector.tensor_scalar(
                        out=t1[:rsz, :], in0=zn[0][:rsz, :],
                        scalar1=dvsb[:rsz, kc, 0:1], scalar2=dvsb[:rsz, kc, 2:3],
                        op0=ALU.mult, op1=ALU.add)
                    t2 = tp.tile([128, TQ], f32, tag="mk2")
                    nc.vector.tensor_scalar(
                        out=t2[:rsz, :], in0=zn[1][:rsz, :],
                        scalar1=dvsb[:rsz, kc, 1:2], scalar2=None, op0=ALU.mult)
                    nc.vector.tensor_sub(t1[:rsz, :], t1[:rsz, :], t2[:rsz, :])
                    nc.scalar.activation(out=t1[:rsz, :], in_=t1[:rsz, :],
                                         func=AF.Sigmoid)
                    o_r = tp.tile([128, TQ], f32, tag="o_r")
                    nc.vector.tensor_mul(o_r[:rsz, :], sqr[:rsz, :], t1[:rsz, :])
                    nc.sync.dma_start(spec2_r.ap()[r0:r0+rsz, :], o_r[:rsz, :])
                    o_rm = tp.tile([128, TQ], f32r, tag="o_rm")
                    nc.vector.tensor_mul(o_rm[:rsz, :], o_r[:rsz, :],
                                         cmask_b[:rsz, :])
                    nc.sync.dma_start(sp2m[0].ap()[r0:r0+rsz, :], o_rm[:rsz, :])
                    t3 = tp.tile([128, TQ], f32, tag="mk3")
                    nc.vector.tensor_scalar(
                        out=t3[:rsz, :], in0=zn[1][:rsz, :],
                        scalar1=dvsb[:rsz, kc, 3:4], scalar2=dvsb[:rsz, kc, 5:6],
                        op0=ALU.mult, op1=ALU.add)
                    t4 = tp.tile([128, TQ], f32, tag="mk4")
                    nc.vector.tensor_scalar(
                        out=t4[:rsz, :], in0=zn[0][:rsz, :],
                        scalar1=dvsb[:rsz, kc, 4:5], scalar2=None, op0=ALU.mult)
                    nc.vector.tensor_add(t3[:rsz, :], t3[:rsz, :], t4[:rsz, :])
                    nc.scalar.activation(out=t3[:rsz, :], in_=t3[:rsz, :],
                                         func=AF.Sigmoid)
                    o_i = tp.tile([128, TQ], f32, tag="o_i")
                    nc.vector.tensor_mul(o_i[:rsz, :], sqi[:rsz, :], t3[:rsz, :])
                    nc.sync.dma_start(spec2_i.ap()[r0:r0+rsz, :], o_i[:rsz, :])
                    o_im = tp.tile([128, TQ], f32r, tag="o_im")
                    nc.vector.tensor_mul(o_im[:rsz, :], o_i[:rsz, :],
                                         cmask_b[:rsz, :])
                    nc.sync.dma_start(sp2m[1].ap()[r0:r0+rsz, :], o_im[:rsz, :])

        # =================== iSTFT + overlap-add =====================
        with tc.tile_pool(name="ist", bufs=1) as istp, \
             tc.tile_pool(name="ist_ps", bufs=4, space="PSUM") as psp, \
             tc.tile_pool(name="ist_w", bufs=6) as wp, \
             tc.tile_pool(name="ist_t", bufs=4) as tp:
            for ch in range(2):
                # load rhs: masked spec rows for this channel, bin-chunked
                rsp = []
                for s in range(2):
                    r_ = istp.tile([128, KC_HD, TQ], f32r, tag=f"rsp{s}")
                    for kc, (r0, rsz) in enumerate(_chunks(BINS)):
                        nc.sync.dma_start(
                            r_[:rsz, kc, :],
                            sp2m[s].ap()[ch*BINS+r0 : ch*BINS+r0+rsz, :])
                    rsp.append(r_)
                bsum = istp.tile([128, 4, SEG_S], f32, tag="bsum")
                nc.vector.memset(bsum[:], 0.0)
                for mn in range(KC_N):
                    pf = psp.tile([128, TQ], f32, tag="pf")
                    ct = wp.tile([128, KC_HD, 128], f32r, tag="ct")
                    nc.sync.dma_start(ct[:], istft_c.ap()[:, mn])
                    st = wp.tile([128, KC_HD, 128], f32r, tag="st")
                    nc.sync.dma_start(st[:], istft_s.ap()[:, mn])
                    for kc, (r0, rsz) in enumerate(_chunks(BINS)):
                        nc.tensor.matmul(pf[:, :], ct[:rsz, kc, :],
                                         rsp[0][:rsz, kc, :],
                                         start=(kc == 0), stop=False)
                        nc.tensor.matmul(pf[:, :], st[:rsz, kc, :],
                                         rsp[1][:rsz, kc, :],
                                         start=False, stop=(kc == KC_HD - 1))
                    j = mn // 4
                    mcb = mn % 4
                    nc.vector.tensor_add(bsum[:, mcb, j:j+TQ], bsum[:, mcb, j:j+TQ],
                                         pf[:, :])
                # transpose bsum -> [s, 512] and write out
                for sc, ssz in _chunks(SEG_S):
                    for mcb in range(4):
                        pst = psp.tile([128, 128], f32, tag="pst")
                        nc.tensor.transpose(pst[:ssz, :], bsum[:, mcb, sc:sc+ssz],
                                            ident[:, :])
                        so = tp.tile([128, 128], f32, tag="so")
                        nc.scalar.copy(so[:ssz, :], pst[:ssz, :])
                        dst = bass.AP(
                            tensor=seg_o.ap().tensor,
                            offset=ch * SEG + sc * 512 + mcb * 128,
                            ap=[[512, ssz], [1, 128]])
                        nc.sync.dma_start(dst, so[:ssz, :])

    nc.compile()
    return nc


MC_D_LAST = 16  # last chunk index of D rows (17 chunks)


# ---------------------------------------------------------------------------
# kernel entry
# ---------------------------------------------------------------------------

_CACHE = {}


def _get_program(debug=False):
    key = ("prog", debug)
    if key not in _CACHE:
        _CACHE[key] = _build_program(debug=debug)
    return _CACHE[key]


def _install_neff_cache():
    """Disk-cache walrus NEFF compiles keyed by BIR hash (compiles are
    10+ minutes; identical BIR -> identical NEFF)."""
    import hashlib
    import shutil
    from concourse import bass_utils, bass2jax
    if getattr(bass_utils, '_neff_cache_installed', False):
        return
    orig = bass_utils.compile_bir_kernel

    def cached(bir_json, tmpdir, neff_name="file.neff"):
        h = hashlib.sha256(bir_json).hexdigest()[:24]
        cdir = os.environ.get("BASS_NEFF_CACHE", "/tmp/bass_neff_cache")
        os.makedirs(cdir, exist_ok=True)
        cpath = os.path.join(cdir, f"{h}_{neff_name}")
        if os.path.exists(cpath):
            dst = os.path.join(tmpdir, neff_name)
            shutil.copy(cpath, dst)
            return dst
        p = orig(bir_json, tmpdir, neff_name=neff_name)
        try:
            shutil.copy(p, cpath)
        except OSError:
            pass
        return p

    bass_utils.compile_bir_kernel = cached
    bass2jax.compile_bir_kernel = cached
    bass_utils._neff_cache_installed = True


def kernel(debug=False, _run_kwargs=None, **inputs):
    from concourse import bass_utils
    _install_neff_cache()

    consts = _prep_constants(inputs)
    wsq = _prep_wsq(inputs['window'])
    mix = np.asarray(inputs['mix'], np.float32)

    pe_own = [np.ascontiguousarray(consts['peT'][:, o:o+TQ]) for o in Q_OFF]
    # half1 overlaps half0 by 2 frames; zero them out of its iSTFT input
    cmask = [np.ones((1, TQ), np.float32), np.ones((1, TQ), np.float32)]
    cmask[1][:, :2] = 0.0

    shared = {k: consts[k] for k in
              ('stft_c', 'stft_s', 'istft_c', 'istft_s', 'c1sc', 'dvec',
               'g1r', 'g1i', 'wq0', 'wq1', 'wk0', 'wk1', 'wv0', 'wv1',
               'wo', 'w1rp', 'w1ip', 'w2rp', 'w2ip')}
    shared['pe_full'] = consts['peT']
    shared['ones_in'] = np.ones((1, T), np.float32)
    shared['zeros_in'] = np.zeros((1, T), np.float32)
    shared['ones32_in'] = np.ones((1, T), np.float32)

    in_maps = []
    for core in range(8):
        b, half = core // 2, core % 2
        apad = np.pad(mix[b], ((0, 0), (PAD, PAD)), mode='reflect')
        m = dict(shared)
        m['audio_full'] = np.ascontiguousarray(apad)
        o = Q_OFF[half] * HOP
        m['audio_own'] = np.ascontiguousarray(apad[:, o:o + SEG])
        m['pe_own'] = pe_own[half]
        m['colmask'] = cmask[half]
        in_maps.append(m)

    prog = _get_program(debug=debug)
    rk = _run_kwargs or {}
    res = bass_utils.run_bass_kernel_spmd(prog, in_maps, core_ids=list(range(8)),
                                          **rk)

    # gather
    p = consts['perm']
    inv = np.empty(D, np.int64); inv[p] = np.arange(D)
    est = np.zeros((B, 2, L), np.float32)
    spec_stack = np.zeros((B * 2, BINS, T, 2), np.float32)
    for b in range(B):
        ola = np.zeros((2, APLEN), np.float64)
        for half in range(2):
            r = res.results[b * 2 + half]
            ola[:, Q_OFF[half]*HOP : Q_OFF[half]*HOP + SEG] += r['seg']
            c0 = 0 if half == 0 else 2
            t0 = Q_OFF[half] + c0
            for ch in range(2):
                spec_stack[b*2+ch, :, t0:Q_OFF[half]+TQ, 0] = \
                    r['spec2_r'][ch*BINS:(ch+1)*BINS, c0:]
                spec_stack[b*2+ch, :, t0:Q_OFF[half]+TQ, 1] = \
                    r['spec2_i'][ch*BINS:(ch+1)*BINS, c0:]
        ola = ola / wsq[None, :]
        est[b] = ola[:, PAD:PAD+L].astype(np.float32)
    if debug:
        kernel.last_results = res
    kernel.last_exec_time_ns = getattr(res, 'exec_time_ns', None)
    return est, spec_stack


# revision 26
# speedup vs baseline: 1.0115x; 1.0115x over previous
"""Trainium2 Bass kernel for nn_AttentionModel_57380763075118.

8 NeuronCores, SPMD, no collectives. core c = (batch b = c//2, half = c%2).
half0 owns frames [0,432), half1 owns [430,862) (both 432 wide -> identical
program; the 2-frame overlap is removed via a per-core column mask folded into
the iSTFT input and by host-side column slicing of the spec output).

Everything on device is feature-major [feature, time]; D is permuted to
ch-major (d = ch*1025 + bin) host-side; biases ride extra contraction rows
(ones-rows); LayerNorm statistics use ones-vector matmuls; per-feature affine
folds are precomputed host-side. Matmuls run in float32r (full PE rate, even
moving dims required -> 432/430 splits, hd padded 1025->1026).
"""
import math
import os
import sys

sys.path.insert(0, '/opt/trn_rl_repo')

import numpy as np

NFFT = 2048; HOP = 512; BINS = 1025
HEADS = 2; D = 2 * BINS; DFF = 4 * D
B = 4; L = 441000
T = 862
HD = 1025
HD_P = 1026
TQ = 432
Q_OFF = (0, T - TQ)            # (0, 430)
PAD = NFFT // 2
AUDLEN = L + 2 * PAD           # 443048 padded audio length
APLEN = NFFT + HOP * (T - 1)   # 442880 overlap-add span
SEG = (TQ - 1) * HOP + NFFT    # 222720 own segment length
NC_T = (432, 430)
KC_D = 17                      # ceil(2051/128)
KC_HD = 9
KC_T = 7
KC_N = 16
MC_DFF = 65                    # ceil(8200/128)
KC_H = 65                      # ceil(8201/128)
NC_HD = (342, 342, 342)
L2_BLK = 7                     # 10 blocks of 7 (last 2) = 65
SEG_S = TQ + 3                 # 435
MC_D = 17                      # ceil(2050/128)


def _chunks(total, size=128):
    return [(i, min(size, total - i)) for i in range(0, total, size)]


# ---------------------------------------------------------------------------
# host-side constant preparation
# ---------------------------------------------------------------------------

def _perm_ch_major():
    p = np.empty(D, np.int64)
    for ch in range(2):
        p[ch * BINS:(ch + 1) * BINS] = np.arange(BINS) * 2 + ch
    return p


def _pos_enc():
    pos = np.arange(T, dtype=np.float32)[:, None]
    div = np.exp(np.arange(0, D, 2, dtype=np.float32) * (-math.log(10000.0) / D))
    pe = np.zeros((T, D), np.float32)
    pe[:, 0::2] = np.sin(pos * div)
    pe[:, 1::2] = np.cos(pos * div)
    return pe


def _prep_constants(inp):
    p = _perm_ch_major()
    window = np.asarray(inp['window'])

    n = np.arange(NFFT, dtype=np.float64)
    k = np.arange(BINS, dtype=np.float64)
    ang = 2.0 * np.pi * np.outer(n, k) / NFFT
    w64 = window.astype(np.float64)[:, None]
    stft_c = (w64 * np.cos(ang)).astype(np.float32)          # [NFFT, BINS]
    stft_s = (-w64 * np.sin(ang)).astype(np.float32)

    ang2 = ang.T                                             # [BINS, NFFT]
    coef = np.full((BINS, 1), 2.0); coef[0] = 1.0; coef[-1] = 1.0
    wrow = window.astype(np.float64)[None, :]
    istft_c = (wrow * coef * np.cos(ang2) / NFFT).astype(np.float32)
    istft_s = (-wrow * coef * np.sin(ang2) / NFFT).astype(np.float32)

    peT = np.ascontiguousarray(_pos_enc()[:, p].T)           # [D, T]

    ch_of = np.arange(D) // BINS
    def chvec(a2):
        return np.asarray(a2, np.float32)[ch_of]

    i = inp
    c1sc = np.zeros((128, 8), np.float32)
    c1wr = np.asarray(i['c1wr']); c1wi = np.asarray(i['c1wi'])
    c1br = np.asarray(i['c1br']); c1bi = np.asarray(i['c1bi'])
    for ch in range(2):
        c1sc[:, 0 + ch] = c1wr[ch]
        c1sc[:, 2 + ch] = c1wi[ch]
        c1sc[:, 4 + ch] = c1br[ch] - c1bi[ch]
        c1sc[:, 6 + ch] = c1br[ch] + c1bi[ch]

    def packd(v):
        vp = np.zeros(KC_D * 128, np.float32); vp[:D] = v
        return np.ascontiguousarray(vp.reshape(KC_D, 128).T)  # [128, KC_D]

    g2r = np.asarray(i['n2_gr'])[p]; b2r_ln = np.asarray(i['n2_br'])[p]
    g2i = np.asarray(i['n2_gi'])[p]; b2i_ln = np.asarray(i['n2_bi'])[p]
    c2wr = chvec(i['c2wr']); c2wi = chvec(i['c2wi'])
    c2br = chvec(i['c2br']); c2bi = chvec(i['c2bi'])
    mvecs = [g2r * c2wr, g2i * c2wi, b2r_ln * c2wr - b2i_ln * c2wi + c2br - c2bi,
             g2i * c2wr, g2r * c2wi, b2i_ln * c2wr + b2r_ln * c2wi + c2br + c2bi]
    dvec = np.stack([packd(v) for v in mvecs], axis=-1)      # [128, KC_D, 6]

    Win = np.asarray(i['attn_in_w']); bin_ = np.asarray(i['attn_in_b'])
    Wq, Wk, Wv = Win[:D], Win[D:2*D], Win[2*D:]
    bq, bk, bv = bin_[:D], bin_[D:2*D], bin_[2*D:]
    scale = 1.0 / np.sqrt(np.float64(HD))
    wqt, wkt, wvt = [], [], []
    for h in range(HEADS):
        for (W, b_, out, sc) in ((Wq, bq, wqt, scale), (Wk, bk, wkt, 1.0),
                                 (Wv, bv, wvt, 1.0)):
            m = np.zeros((D + 1, HD_P), np.float32)
            m[:D, :HD] = (W[h*HD:(h+1)*HD][:, p].T * sc).astype(np.float32)
            m[D, :HD] = (np.asarray(b_[h*HD:(h+1)*HD]) * sc).astype(np.float32)
            out.append(np.ascontiguousarray(m))

    Wo = np.asarray(i['attn_out_w']); bo = np.asarray(i['attn_out_b'])
    wo = np.zeros((2 * HD_P + 1, D), np.float32)
    for h in range(HEADS):
        wo[h*HD_P : h*HD_P + HD] = Wo[:, h*HD:(h+1)*HD][p, :].T
    wo[2 * HD_P] = 2.0 * bo[p]

    # FFN l1 [D+1, DFF]; LN1 gains are applied on device (per-row tensor_scalar)
    W1r = np.asarray(i['l1wr']); W1i = np.asarray(i['l1wi'])
    b1r = np.asarray(i['l1br']); b1i = np.asarray(i['l1bi'])
    g1r = np.asarray(i['n1_gr'])[p]; b1r_ln = np.asarray(i['n1_br'])[p]
    g1i = np.asarray(i['n1_gi'])[p]; b1i_ln = np.asarray(i['n1_bi'])[p]
    W1rT = W1r[:, p].T; W1iT = W1i[:, p].T                   # [D, DFF]
    w1r = np.zeros((D + 1, DFF), np.float32)
    w1i = np.zeros((D + 1, DFF), np.float32)
    w1r[:D] = W1rT
    w1i[:D] = W1iT
    # bias rows (ride ynr' ones-row): include LN1 beta folds
    w1r[D] = (b1r - b1i) + b1r_ln @ W1rT - b1i_ln @ W1iT
    w1i[D] = (b1r + b1i) + b1i_ln @ W1rT + b1r_ln @ W1iT

    W2r = np.asarray(i['l2wr']); W2i = np.asarray(i['l2wi'])
    b2r = np.asarray(i['l2br']); b2i = np.asarray(i['l2bi'])
    w2r = np.zeros((DFF + 1, D), np.float32)
    w2i = np.zeros((DFF + 1, D), np.float32)
    w2r[:DFF] = W2r[p, :].T
    w2i[:DFF] = W2i[p, :].T
    w2r[DFF] = (b2r - b2i)[p]
    w2i[DFF] = (b2r + b2i)[p]

    def pretile(w, kc_n, mc_n):
        rows = kc_n * 128; cols = mc_n * 128
        wp_ = np.zeros((rows, cols), np.float32)
        wp_[:w.shape[0], :w.shape[1]] = w
        # [kc, p, mc, m] -> [p, mc, kc, m]; contiguous (kc, m) per (p, mc)
        return np.ascontiguousarray(
            wp_.reshape(kc_n, 128, mc_n, 128).transpose(1, 2, 0, 3))

    wqp = [pretile(w, 17, 9) for w in wqt]
    wkp = [pretile(w, 17, 9) for w in wkt]
    wop = pretile(wo, 17, 17)
    stft_cp = pretile(stft_c, 16, 9)
    stft_sp = pretile(stft_s, 16, 9)
    istft_cp = pretile(istft_c, 9, 16)
    istft_sp = pretile(istft_s, 9, 16)
    w1rp = pretile(w1r, 17, 65)
    w1ip = pretile(w1i, 17, 65)
    w2rp = pretile(w2r, 65, 17)
    w2ip = pretile(w2i, 65, 17)

    return dict(w1rp=w1rp, w1ip=w1ip, w2rp=w2rp, w2ip=w2ip,
                wq0=wqp[0], wq1=wqp[1], wk0=wkp[0], wk1=wkp[1], wo=wop,
                stft_c=stft_cp, stft_s=stft_sp,
                istft_c=istft_cp, istft_s=istft_sp,
                peT=peT, c1sc=c1sc, dvec=dvec,
                g1r=packd(g1r), g1i=packd(g1i),
                wv0=wvt[0], wv1=wvt[1],
                w1r=w1r, w1i=w1i, w2r=w2r, w2i=w2i, perm=p)


def _prep_wsq(window):
    win = np.asarray(window, np.float64)
    idx = (np.arange(T)[:, None] * HOP + np.arange(NFFT)[None, :]).ravel()
    wsq = np.zeros(APLEN)
    np.add.at(wsq, idx, np.tile(win ** 2, T))
    return np.where(wsq > 1e-11, wsq, 1.0)


# ---------------------------------------------------------------------------
# bass program
# ---------------------------------------------------------------------------

def _build_program(debug=False, precise_qk=True, precise_stft=True):
    import concourse.bass as bass
    import concourse.bacc as bacc
    import concourse.tile as tile
    from concourse import mybir
    from concourse.masks import make_identity

    f32 = mybir.dt.float32
    f32r = mybir.dt.float32r
    AF = mybir.ActivationFunctionType
    ALU = mybir.AluOpType
    AX = mybir.AxisListType

    nc = bacc.Bacc("TRN2", target_bir_lowering=False, debug=False, num_devices=8)

    def din(name, shape, dt=f32r):
        return nc.dram_tensor(name, shape, dt, kind="ExternalInput")

    audio_full = din("audio_full", (2, AUDLEN), f32)
    audio_own = din("audio_own", (2, SEG), f32)
    pe_full = din("pe_full", (D, T), f32)
    pe_own = din("pe_own", (D, TQ), f32)
    c1sc_t = din("c1sc", (128, 8), f32)
    dvec_t = din("dvec", (128, KC_D, 6), f32)
    g1r_t = din("g1r", (128, KC_D), f32)
    g1i_t = din("g1i", (128, KC_D), f32)
    colmask_t = din("colmask", (1, TQ), f32)
    ones_in = din("ones_in", (1, T), f32r)
    zeros_in = din("zeros_in", (1, T), f32r)
    ones32_in = din("ones32_in", (1, T), f32)
    st_dt = f32 if precise_stft else f32r
    stft_c = din("stft_c", (128, 9, KC_N, 128), st_dt)
    stft_s = din("stft_s", (128, 9, KC_N, 128), st_dt)
    istft_c = din("istft_c", (128, KC_N, KC_HD, 128))
    istft_s = din("istft_s", (128, KC_N, KC_HD, 128))
    qk_dt = f32 if precise_qk else f32r
    wq = [din("wq0", (128, KC_HD, KC_D, 128), qk_dt),
          din("wq1", (128, KC_HD, KC_D, 128), qk_dt)]
    wk = [din("wk0", (128, KC_HD, KC_D, 128), qk_dt),
          din("wk1", (128, KC_HD, KC_D, 128), qk_dt)]
    wv = [din("wv0", (D + 1, HD_P)), din("wv1", (D + 1, HD_P))]
    wo_t = din("wo", (128, MC_D, KC_D, 128))
    w1r_t = din("w1rp", (128, MC_DFF, KC_D, 128))
    w1i_t = din("w1ip", (128, MC_DFF, KC_D, 128))
    w2r_t = din("w2rp", (128, MC_D, KC_H, 128))
    w2i_t = din("w2ip", (128, MC_D, KC_H, 128))

    spec2_r = nc.dram_tensor("spec2_r", (D, TQ), f32, kind="ExternalOutput")
    spec2_i = nc.dram_tensor("spec2_i", (D, TQ), f32, kind="ExternalOutput")
    seg_o = nc.dram_tensor("seg", (2, SEG), f32, kind="ExternalOutput")

    def dscratch(name, shape, dt=f32r):
        kind = "ExternalOutput" if debug else "Internal"
        return nc.dram_tensor(name, shape, dt, kind=kind)

    xq_d = [dscratch("xq_r", (D + 1, TQ)), dscratch("xq_i", (D + 1, TQ))]
    xt_d = [dscratch("xt_r", (D + 1, T)), dscratch("xt_i", (D + 1, T))]
    # f32 copies of x for the precise q/k path (softmax logits need fp32)
    xq32_d = [dscratch("xq32_r", (D + 1, TQ), f32),
              dscratch("xq32_i", (D + 1, TQ), f32)] if precise_qk else xq_d
    xt32_d = [dscratch("xt32_r", (D + 1, T), f32),
              dscratch("xt32_i", (D + 1, T), f32)] if precise_qk else xt_d
    specq_d = [dscratch("specq_r", (D, TQ), f32), dscratch("specq_i", (D, TQ), f32)]
    qt_d = [[dscratch(f"qt_{s}{h}", (HD_P, TQ), qk_dt) for s in "ri"]
            for h in range(2)]
    kt_d = [[dscratch(f"kt_{s}{h}", (HD_P, T), qk_dt) for s in "ri"]
            for h in range(2)]
    v_d = [[dscratch(f"v_{s}{h}", (T, HD_P)) for s in "ri"] for h in range(2)]
    ot_d = [dscratch("ot_r", (2 * HD_P + 1, TQ)),
            dscratch("ot_i", (2 * HD_P + 1, TQ))]
    h_d = [dscratch("h_r", (DFF + 1, TQ)), dscratch("h_i", (DFF + 1, TQ)),
           dscratch("h_in", (DFF + 1, TQ))]
    y_d = [dscratch("y_r", (D, TQ), f32), dscratch("y_i", (D, TQ), f32)]
    z_tap = [dscratch("z_r", (D, TQ), f32), dscratch("z_i", (D, TQ), f32)] if debug else None
    bc_d = nc.dram_tensor("bcast_scratch", (8, TQ), f32, kind="Internal")

    with tile.TileContext(nc) as tc:
      with tc.tile_pool(name="const", bufs=1) as constp:
        ident = constp.tile([128, 128], f32)
        make_identity(nc, ident[:])
        ones_col = constp.tile([128, 1], f32)
        nc.vector.memset(ones_col[:], 1.0)
        eps_t = constp.tile([1, 1], f32)
        nc.vector.memset(eps_t[:], 1e-5)
        c1sb = constp.tile([128, 8], f32)
        nc.sync.dma_start(c1sb[:], c1sc_t.ap())
        dvsb = constp.tile([128, KC_D, 6], f32)
        nc.sync.dma_start(dvsb[:], dvec_t.ap())
        g1rsb = constp.tile([128, KC_D], f32)
        nc.sync.dma_start(g1rsb[:], g1r_t.ap())
        g1isb = constp.tile([128, KC_D], f32)
        nc.sync.dma_start(g1isb[:], g1i_t.ap())
        cmask_b = constp.tile([128, TQ], f32)
        nc.sync.dma_start(cmask_b[:], bass.AP(
            tensor=colmask_t.ap().tensor, offset=0, ap=[[0, 128], [1, TQ]]))

        # ones rows for DRAM scratch (DRAM->DRAM)
        nc.sync.dma_start(xq_d[0].ap()[D:D+1, :], ones_in.ap()[:, :TQ])
        nc.sync.dma_start(xq_d[1].ap()[D:D+1, :], ones_in.ap()[:, :TQ])
        nc.sync.dma_start(xt_d[0].ap()[D:D+1, :], ones_in.ap()[:, :T])
        nc.sync.dma_start(xt_d[1].ap()[D:D+1, :], ones_in.ap()[:, :T])
        if precise_qk:
            nc.sync.dma_start(xq32_d[0].ap()[D:D+1, :], ones32_in.ap()[:, :TQ])
            nc.sync.dma_start(xq32_d[1].ap()[D:D+1, :], ones32_in.ap()[:, :TQ])
            nc.sync.dma_start(xt32_d[0].ap()[D:D+1, :], ones32_in.ap()[:, :T])
            nc.sync.dma_start(xt32_d[1].ap()[D:D+1, :], ones32_in.ap()[:, :T])
        nc.sync.dma_start(ot_d[0].ap()[2*HD_P:2*HD_P+1, :], zeros_in.ap()[:, :TQ])
        nc.sync.dma_start(ot_d[1].ap()[2*HD_P:2*HD_P+1, :], ones_in.ap()[:, :TQ])
        nc.sync.dma_start(h_d[0].ap()[DFF:DFF+1, :], ones_in.ap()[:, :TQ])
        nc.sync.dma_start(h_d[1].ap()[DFF:DFF+1, :], zeros_in.ap()[:, :TQ])
        nc.sync.dma_start(h_d[2].ap()[DFF:DFF+1, :], zeros_in.ap()[:, :TQ])

        # =================== STFT stage helper ===========================
        def stft_stage(audio_t, width, nc_chunks, pe_t, x_out, x32_out, specq_out):
            with tc.tile_pool(name="fw", bufs=1) as fwp:
                fwT = [fwp.tile([128, KC_N, width], st_dt, tag=f"fwT{c}", name=f"fwT{c}")
                       for c in range(2)]
                with tc.tile_pool(name="fr", bufs=2) as frp, \
                     tc.tile_pool(name="tr_ps", bufs=6, space="PSUM") as trps:
                    for ch in range(2):
                        for t0, tsz in _chunks(width):
                            fr_t = frp.tile([128, NFFT], f32, tag="fr")
                            src = bass.AP(
                                tensor=audio_t.ap().tensor,
                                offset=ch * audio_t.shape[1] + t0 * HOP,
                                ap=[[HOP, tsz], [1, NFFT]])
                            nc.sync.dma_start(fr_t[:tsz, :], src)
                            for nch in range(KC_N):
                                pst = trps.tile([128, 128], f32, tag="tr")
                                nc.tensor.transpose(
                                    pst[:, :tsz],
                                    fr_t[:tsz, nch*128:(nch+1)*128],
                                    ident[:tsz, :tsz])
                                nc.scalar.copy(fwT[ch][:, nch, t0:t0+tsz],
                                               pst[:, :tsz])
                with tc.tile_pool(name="st_ps", bufs=1, space="PSUM") as psp, \
                     tc.tile_pool(name="st_w", bufs=2) as wp, \
                     tc.tile_pool(name="st_t", bufs=2) as tp:
                    for mci, (mc, msz) in enumerate(_chunks(BINS)):
                        pre = [[psp.tile([128, 512], f32, tag=f"re{c}{j}", name=f"re{c}{j}")
                                for j in range(len(nc_chunks))] for c in range(2)]
                        pim = [[psp.tile([128, 512], f32, tag=f"im{c}{j}", name=f"im{c}{j}")
                                for j in range(len(nc_chunks))] for c in range(2)]
                        ct = wp.tile([128, KC_N, 128], st_dt, tag="ct")
                        nc.sync.dma_start(ct[:], stft_c.ap()[:, mci])
                        st = wp.tile([128, KC_N, 128], st_dt, tag="st")
                        nc.sync.dma_start(st[:], stft_s.ap()[:, mci])
                        for kc in range(KC_N):
                            off = 0
                            for j, nsz in enumerate(nc_chunks):
                                for c in range(2):
                                    nc.tensor.matmul(
                                        pre[c][j][:msz, :nsz], ct[:, kc, :msz],
                                        fwT[c][:, kc, off:off+nsz],
                                        start=(kc == 0), stop=(kc == KC_N - 1))
                                    nc.tensor.matmul(
                                        pim[c][j][:msz, :nsz], st[:, kc, :msz],
                                        fwT[c][:, kc, off:off+nsz],
                                        start=(kc == 0), stop=(kc == KC_N - 1))
                                off += nsz
                        for c in range(2):
                            row = c * BINS + mc
                            off = 0
                            for j, nsz in enumerate(nc_chunks):
                                ps_re = pre[c][j][:msz, :nsz]
                                ps_im = pim[c][j][:msz, :nsz]
                                if specq_out is not None:
                                    sq = tp.tile([128, 512], f32, tag="sq")
                                    nc.scalar.copy(sq[:msz, :nsz], ps_re)
                                    nc.sync.dma_start(
                                        specq_out[0].ap()[row:row+msz, off:off+nsz],
                                        sq[:msz, :nsz])
                                    sq2 = tp.tile([128, 512], f32, tag="sq2")
                                    nc.scalar.copy(sq2[:msz, :nsz], ps_im)
                                    nc.sync.dma_start(
                                        specq_out[1].ap()[row:row+msz, off:off+nsz],
                                        sq2[:msz, :nsz])
                                pe_sb = tp.tile([128, 512], f32, tag="pe")
                                nc.sync.dma_start(
                                    pe_sb[:msz, :nsz],
                                    pe_t.ap()[row:row+msz, off:off+nsz])
                                t1 = tp.tile([128, 512], f32, tag="t1")
                                nc.vector.tensor_scalar(
                                    out=t1[:msz, :nsz], in0=ps_re,
                                    scalar1=c1sb[:msz, 0+c:1+c],
                                    scalar2=c1sb[:msz, 4+c:5+c],
                                    op0=ALU.mult, op1=ALU.add)
                                t2 = tp.tile([128, 512], f32, tag="t2")
                                nc.vector.tensor_scalar(
                                    out=t2[:msz, :nsz], in0=ps_im,
                                    scalar1=c1sb[:msz, 2+c:3+c], scalar2=None,
                                    op0=ALU.mult)
                                nc.vector.tensor_sub(t1[:msz, :nsz],
                                                     t1[:msz, :nsz], t2[:msz, :nsz])
                                xo32 = tp.tile([128, 512], f32, tag="xo32")
                                nc.vector.tensor_add(xo32[:msz, :nsz],
                                                     t1[:msz, :nsz], pe_sb[:msz, :nsz])
                                xo = tp.tile([128, 512], f32r, tag="xo")
                                nc.scalar.copy(xo[:msz, :nsz], xo32[:msz, :nsz])
                                nc.sync.dma_start(
                                    x_out[0].ap()[row:row+msz, off:off+nsz],
                                    xo[:msz, :nsz])
                                if precise_qk:
                                    nc.sync.dma_start(
                                        x32_out[0].ap()[row:row+msz, off:off+nsz],
                                        xo32[:msz, :nsz])
                                t3 = tp.tile([128, 512], f32, tag="t3")
                                nc.vector.tensor_scalar(
                                    out=t3[:msz, :nsz], in0=ps_im,
                                    scalar1=c1sb[:msz, 0+c:1+c],
                                    scalar2=c1sb[:msz, 6+c:7+c],
                                    op0=ALU.mult, op1=ALU.add)
                                t4 = tp.tile([128, 512], f32, tag="t4")
                                nc.vector.tensor_scalar(
                                    out=t4[:msz, :nsz], in0=ps_re,
                                    scalar1=c1sb[:msz, 2+c:3+c], scalar2=None,
                                    op0=ALU.mult)
                                nc.vector.tensor_add(t3[:msz, :nsz],
                                                     t3[:msz, :nsz], t4[:msz, :nsz])
                                xo232 = tp.tile([128, 512], f32, tag="xo232")
                                nc.vector.tensor_add(xo232[:msz, :nsz],
                                                     t3[:msz, :nsz], pe_sb[:msz, :nsz])
                                xo2 = tp.tile([128, 512], f32r, tag="xo2")
                                nc.scalar.copy(xo2[:msz, :nsz], xo232[:msz, :nsz])
                                nc.sync.dma_start(
                                    x_out[1].ap()[row:row+msz, off:off+nsz],
                                    xo2[:msz, :nsz])
                                if precise_qk:
                                    nc.sync.dma_start(
                                        x32_out[1].ap()[row:row+msz, off:off+nsz],
                                        xo232[:msz, :nsz])
                                off += nsz

        # =================== stage A: own STFT ===========================
        stft_stage(audio_own, TQ, (TQ,), pe_own, xq_d, xq32_d, specq_d)

        # =================== in_proj Q ===================================
        with tc.tile_pool(name="xq", bufs=1) as xqp:
            xq_sb = []
            for s in range(2):
                xt_ = xqp.tile([128, KC_D, TQ], qk_dt, tag=f"xq{s}")
                for kc, (r0, rsz) in enumerate(_chunks(D + 1)):
                    nc.sync.dma_start(xt_[:rsz, kc, :], xq32_d[s].ap()[r0:r0+rsz, :])
                xq_sb.append(xt_)
            with tc.tile_pool(name="q_ps", bufs=2, space="PSUM") as psp, \
                 tc.tile_pool(name="q_w", bufs=4) as wp, \
                 tc.tile_pool(name="q_o", bufs=4) as op_:
                for h in range(2):
                    for mci, (mc, msz) in enumerate(_chunks(HD_P)):
                        pq = [psp.tile([128, TQ], f32, tag=f"pq{s}", name=f"pq{s}")
                              for s in range(2)]
                        wt = wp.tile([128, KC_D, 128], qk_dt, tag="wt")
                        nc.sync.dma_start(wt[:], wq[h].ap()[:, mci])
                        for kc, (r0, rsz) in enumerate(_chunks(D + 1)):
                            for s in range(2):
                                nc.tensor.matmul(pq[s][:msz, :],
                                                 wt[:rsz, kc, :msz],
                                                 xq_sb[s][:rsz, kc, :],
                                                 start=(kc == 0),
                                                 stop=(kc == KC_D - 1))
                        for s in range(2):
                            ot = op_.tile([128, TQ], qk_dt, tag="qo")
                            nc.scalar.copy(ot[:msz, :], pq[s][:msz, :])
                            nc.sync.dma_start(qt_d[h][s].ap()[mc:mc+msz, :],
                                              ot[:msz, :])

        # =================== stage B: full STFT ==========================
        stft_stage(audio_full, T, NC_T, pe_full, xt_d, xt32_d, None)

        # =================== in_proj K and V =============================
        with tc.tile_pool(name="xtk", bufs=1) as xtkp:
            xt32_sb = []     # qk-dtype x (for K)
            for s in range(2):
                xt3 = xtkp.tile([128, KC_D, T], qk_dt, tag=f"xt32{s}",
                                name=f"xt32{s}")
                for kc, (r0, rsz) in enumerate(_chunks(D + 1)):
                    nc.sync.dma_start(xt3[:rsz, kc, :],
                                      xt32_d[s].ap()[r0:r0+rsz, :])
                xt32_sb.append(xt3)
            with tc.tile_pool(name="k_ps", bufs=2, space="PSUM") as psp, \
                 tc.tile_pool(name="k_w", bufs=4) as wp, \
                 tc.tile_pool(name="k_o", bufs=4) as op_:
                for h in range(2):
                    for mci, (mc, msz) in enumerate(_chunks(HD_P)):
                        pk = [[psp.tile([128, 512], f32, tag=f"pk{s}{j}", name=f"pk{s}{j}")
                               for j in range(2)] for s in range(2)]
                        wt = wp.tile([128, KC_D, 128], qk_dt, tag="wt")
                        nc.sync.dma_start(wt[:], wk[h].ap()[:, mci])
                        for kc, (r0, rsz) in enumerate(_chunks(D + 1)):
                            off = 0
                            for j, nsz in enumerate(NC_T):
                                for s in range(2):
                                    nc.tensor.matmul(
                                        pk[s][j][:msz, :nsz], wt[:rsz, kc, :msz],
                                        xt32_sb[s][:rsz, kc, off:off+nsz],
                                        start=(kc == 0), stop=(kc == KC_D - 1))
                                off += nsz
                        off = 0
                        for j, nsz in enumerate(NC_T):
                            for s in range(2):
                                ot = op_.tile([128, 512], qk_dt, tag="ko")
                                nc.scalar.copy(ot[:msz, :nsz], pk[s][j][:msz, :nsz])
                                nc.sync.dma_start(
                                    kt_d[h][s].ap()[mc:mc+msz, off:off+nsz],
                                    ot[:msz, :nsz])
                            off += nsz
        with tc.tile_pool(name="xtv", bufs=1) as xtvp:
            xt_sb = []
            for s in range(2):
                xt_ = xtvp.tile([128, KC_D, T], f32r, tag=f"xt{s}")
                for kc, (r0, rsz) in enumerate(_chunks(D + 1)):
                    nc.sync.dma_start(xt_[:rsz, kc, :], xt_d[s].ap()[r0:r0+rsz, :])
                xt_sb.append(xt_)
            with tc.tile_pool(name="v_ps", bufs=3, space="PSUM") as psp, \
                 tc.tile_pool(name="v_w", bufs=1) as wp, \
                 tc.tile_pool(name="v_o", bufs=4) as op_:
                for h in range(2):
                    woff = 0
                    for nsz in NC_HD:
                        wv_sb = wp.tile([128, KC_D, nsz], f32r, tag="wv_sb")
                        for kc, (r0, rsz) in enumerate(_chunks(D + 1)):
                            nc.sync.dma_start(
                                wv_sb[:rsz, kc, :],
                                wv[h].ap()[r0:r0+rsz, woff:woff+nsz])
                        for s in range(2):
                            for tc_, tsz in _chunks(T):
                                pv = psp.tile([128, 512], f32, tag="pv")
                                for kc, (r0, rsz) in enumerate(_chunks(D + 1)):
                                    nc.tensor.matmul(
                                        pv[:tsz, :nsz],
                                        xt_sb[s][:rsz, kc, tc_:tc_+tsz],
                                        wv_sb[:rsz, kc, :],
                                        start=(kc == 0), stop=(kc == KC_D - 1))
                                vo = op_.tile([128, 512], f32r, tag="vo")
                                nc.scalar.copy(vo[:tsz, :nsz], pv[:tsz, :nsz])
                                nc.sync.dma_start(
                                    v_d[h][s].ap()[tc_:tc_+tsz, woff:woff+nsz],
                                    vo[:tsz, :nsz])
                        woff += nsz

        # =================== attention per head ==========================
        for h in range(2):
            with tc.tile_pool(name="at_pmn", bufs=1) as mp_:
                pmn_sb = mp_.tile([128, 3, KC_T, TQ], f32r)
                with tc.tile_pool(name="at_qk", bufs=1) as qkp, \
                     tc.tile_pool(name="at_ps", bufs=2, space="PSUM") as psp, \
                     tc.tile_pool(name="tr_ps2", bufs=4, space="PSUM") as trps, \
                     tc.tile_pool(name="at_e", bufs=1) as ep, \
                     tc.tile_pool(name="at_t", bufs=2) as tp:
                    qt_sb, kt_sb = [], []
                    for s in range(2):
                        q_ = qkp.tile([128, KC_HD, TQ], qk_dt, tag=f"q{s}")
                        for kc, (r0, rsz) in enumerate(_chunks(HD_P)):
                            nc.sync.dma_start(q_[:rsz, kc, :],
                                              qt_d[h][s].ap()[r0:r0+rsz, :])
                        qt_sb.append(q_)
                        k_ = qkp.tile([128, KC_HD, T], qk_dt, tag=f"k{s}")
                        for kc, (r0, rsz) in enumerate(_chunks(HD_P)):
                            nc.sync.dma_start(k_[:rsz, kc, :],
                                              kt_d[h][s].ap()[r0:r0+rsz, :])
                        kt_sb.append(k_)

                    def softmax_map(a, b_, etag):
                        e_t = ep.tile([128, 4, T], f32, tag=etag)
                        rden = tp.tile([128, 4, 1], f32, tag=f"rd{etag}")
                        for mqi, (q0, qsz) in enumerate(_chunks(TQ)):
                            ps = [psp.tile([128, 512], f32, tag=f"s{j}", name=f"sps{j}")
                                  for j in range(2)]
                            for kc, (r0, rsz) in enumerate(_chunks(HD_P)):
                                off = 0
                                for j, nsz in enumerate(NC_T):
                                    nc.tensor.matmul(
                                        ps[j][:qsz, :nsz],
                                        qt_sb[a][:rsz, kc, q0:q0+qsz],
                                        kt_sb[b_][:rsz, kc, off:off+nsz],
                                        start=(kc == 0), stop=(kc == KC_HD - 1))
                                    off += nsz
                            mx = tp.tile([128, 2], f32, tag="mx")
                            nc.vector.reduce_max(mx[:qsz, 0:1],
                                                 ps[0][:qsz, :NC_T[0]], axis=AX.X)
                            nc.vector.reduce_max(mx[:qsz, 1:2],
                                                 ps[1][:qsz, :NC_T[1]], axis=AX.X)
                            nmx = tp.tile([128, 1], f32, tag="nmx")
                            nc.vector.reduce_max(nmx[:qsz, :], mx[:qsz, :], axis=AX.X)
                            nc.vector.tensor_scalar_mul(nmx[:qsz, :],
                                                        nmx[:qsz, :], -1.0)
                            dn = tp.tile([128, 2], f32, tag="dn")
                            off = 0
                            for j, nsz in enumerate(NC_T):
                                nc.scalar.activation(
                                    out=e_t[:qsz, mqi, off:off+nsz],
                                    in_=ps[j][:qsz, :nsz], func=AF.Exp,
                                    bias=nmx[:qsz, :], scale=1.0,
                                    accum_out=dn[:qsz, j:j+1])
                                off += nsz
                            nc.vector.tensor_add(dn[:qsz, 0:1], dn[:qsz, 0:1],
                                                 dn[:qsz, 1:2])
                            nc.vector.reciprocal(rden[:qsz, mqi, :], dn[:qsz, 0:1])
                        return e_t, rden

                    def combine_transpose(ea, rda, eb, rdb, dst_idx, sub):
                        for mqi, (q0, qsz) in enumerate(_chunks(TQ)):
                            t1 = tp.tile([128, T], f32, tag="c1")
                            nc.vector.tensor_scalar(
                                out=t1[:qsz, :], in0=ea[:qsz, mqi, :],
                                scalar1=rda[:qsz, mqi, :], scalar2=None,
                                op0=ALU.mult)
                            t2 = tp.tile([128, T], f32, tag="c2")
                            nc.vector.tensor_scalar(
                                out=t2[:qsz, :], in0=eb[:qsz, mqi, :],
                                scalar1=rdb[:qsz, mqi, :], scalar2=None,
                                op0=ALU.mult)
                            cm = tp.tile([128, T], f32, tag="cm")
                            if sub:
                                nc.vector.tensor_sub(cm[:qsz, :], t1[:qsz, :],
                                                     t2[:qsz, :])
                            else:
                                nc.vector.tensor_add(cm[:qsz, :], t1[:qsz, :],
                                                     t2[:qsz, :])
                            for kci, (k0, ksz) in enumerate(_chunks(T)):
                                pst = trps.tile([128, 128], f32, tag="trm")
                                nc.tensor.transpose(pst[:ksz, :qsz],
                                                    cm[:qsz, k0:k0+ksz],
                                                    ident[:qsz, :qsz])
                                nc.scalar.copy(
                                    pmn_sb[:ksz, dst_idx, kci, q0:q0+qsz],
                                    pst[:ksz, :qsz])

                    e_rr, rd_rr = softmax_map(0, 0, "eA")
                    e_ii, rd_ii = softmax_map(1, 1, "eB")
                    combine_transpose(e_rr, rd_rr, e_ii, rd_ii, 0, True)   # P
                    e_ri, rd_ri = softmax_map(0, 1, "eA")
                    e_ir, rd_ir = softmax_map(1, 0, "eB")
                    combine_transpose(e_ri, rd_ri, e_ir, rd_ir, 1, True)   # M
                    combine_transpose(e_ri, rd_ri, e_ir, rd_ir, 2, False)  # N

                # applies: oT_r = v_r.T@P + v_i.T@M ; oT_i = v_i.T@P + v_r.T@N
                with tc.tile_pool(name="at_v", bufs=1) as vp, \
                     tc.tile_pool(name="ap_ps", bufs=4, space="PSUM") as psp, \
                     tc.tile_pool(name="ap_o", bufs=4) as op_:
                    v_sb = []
                    for s in range(2):
                        v_ = vp.tile([128, KC_T, HD_P], f32r, tag=f"v{s}")
                        for kc, (r0, rsz) in enumerate(_chunks(T)):
                            nc.sync.dma_start(v_[:rsz, kc, :],
                                              v_d[h][s].ap()[r0:r0+rsz, :])
                        v_sb.append(v_)
                    for si, (va, ma, vb, mb) in enumerate(
                            ((0, 0, 1, 1), (1, 0, 0, 2))):
                        for mc, msz in _chunks(HD_P):
                            pa = psp.tile([128, TQ], f32, tag="pa")
                            for kc, (r0, rsz) in enumerate(_chunks(T)):
                                nc.tensor.matmul(
                                    pa[:msz, :], v_sb[va][:rsz, kc, mc:mc+msz],
                                    pmn_sb[:rsz, ma, kc, :],
                                    start=(kc == 0), stop=False)
                            for kc, (r0, rsz) in enumerate(_chunks(T)):
                                nc.tensor.matmul(
                                    pa[:msz, :], v_sb[vb][:rsz, kc, mc:mc+msz],
                                    pmn_sb[:rsz, mb, kc, :],
                                    start=False, stop=(kc == KC_T - 1))
                            oo = op_.tile([128, TQ], f32r, tag="oo")
                            nc.scalar.copy(oo[:msz, :], pa[:msz, :])
                            nc.sync.dma_start(
                                ot_d[si].ap()[h*HD_P+mc : h*HD_P+mc+msz, :],
                                oo[:msz, :])

        # =================== out_proj (+ fused LN1 stats) ================
        with tc.tile_pool(name="op_ot", bufs=1) as otp, \
             tc.tile_pool(name="op_ps", bufs=2, space="PSUM") as psp, \
             tc.tile_pool(name="st_ps2", bufs=1, space="PSUM") as stp, \
             tc.tile_pool(name="op_w", bufs=2) as wp, \
             tc.tile_pool(name="op_t", bufs=4) as otmp:
            ot_sb = []
            for s in range(2):
                o_ = otp.tile([128, KC_D, TQ], f32r, tag=f"ot{s}", name=f"ot{s}")
                for kc, (r0, rsz) in enumerate(_chunks(2 * HD_P + 1)):
                    nc.sync.dma_start(o_[:rsz, kc, :], ot_d[s].ap()[r0:r0+rsz, :])
                ot_sb.append(o_)
            pm = [stp.tile([1, TQ], f32, tag=f"pm{s}", name=f"pm{s}")
                  for s in range(2)]
            pv = [stp.tile([1, TQ], f32, tag=f"pv{s}", name=f"pv{s}")
                  for s in range(2)]
            for mci, (mc, msz) in enumerate(_chunks(D)):
                py = [psp.tile([128, TQ], f32, tag=f"py{s}", name=f"py{s}")
                      for s in range(2)]
                wt = wp.tile([128, KC_D, 128], f32r, tag="wt")
                nc.sync.dma_start(wt[:], wo_t.ap()[:, mci])
                for kc, (r0, rsz) in enumerate(_chunks(2 * HD_P + 1)):
                    for s in range(2):
                        nc.tensor.matmul(py[s][:msz, :], wt[:rsz, kc, :msz],
                                         ot_sb[s][:rsz, kc, :],
                                         start=(kc == 0), stop=(kc == KC_D - 1))
                for s in range(2):
                    yc = otmp.tile([128, TQ], f32, tag=f"yc{s}")
                    nc.scalar.copy(yc[:msz, :], py[s][:msz, :])
                    nc.sync.dma_start(y_d[s].ap()[mc:mc+msz, :], yc[:msz, :])
                    nc.tensor.matmul(pm[s][:1, :], ones_col[:msz, :], yc[:msz, :],
                                     start=(mci == 0), stop=(mci == MC_D_LAST))
                    sq = otmp.tile([128, TQ], f32, tag=f"sq{s}")
                    nc.scalar.activation(out=sq[:msz, :], in_=yc[:msz, :],
                                         func=AF.Square)
                    nc.tensor.matmul(pv[s][:1, :], ones_col[:msz, :], sq[:msz, :],
                                     start=(mci == 0), stop=(mci == MC_D_LAST))
            # finalize LN1 stats -> DRAM-bounce broadcast tiles kept in a
            # small long-lived pool for the normalize pass below
            with tc.tile_pool(name="ln_bc", bufs=1) as bcp:
                stats = []
                for s in range(2):
                    mT = otmp.tile([1, TQ], f32, tag=f"m{s}")
                    nc.scalar.mul(mT[:1, :], pm[s][:1, :], 1.0 / D)
                    msq = otmp.tile([1, TQ], f32, tag=f"msq{s}")
                    nc.scalar.activation(out=msq[:1, :], in_=mT[:1, :],
                                         func=AF.Square)
                    var = otmp.tile([1, TQ], f32, tag=f"var{s}")
                    nc.scalar.mul(var[:1, :], pv[s][:1, :], 1.0 / D)
                    nc.vector.tensor_sub(var[:1, :], var[:1, :], msq[:1, :])
                    nc.scalar.activation(out=var[:1, :], in_=var[:1, :],
                                         func=AF.Sqrt, bias=eps_t[:1, :])
                    rs = otmp.tile([1, TQ], f32, tag=f"rs{s}")
                    nc.vector.reciprocal(rs[:1, :], var[:1, :])
                    nc.sync.dma_start(bc_d.ap()[2*s:2*s+1, :], mT[:1, :])
                    nc.sync.dma_start(bc_d.ap()[2*s+1:2*s+2, :], rs[:1, :])
                    mb = bcp.tile([128, TQ], f32, tag=f"mb{s}", name=f"mb{s}")
                    nc.sync.dma_start(mb[:], bass.AP(
                        tensor=bc_d.ap().tensor, offset=2*s*TQ,
                        ap=[[0, 128], [1, TQ]]))
                    rb = bcp.tile([128, TQ], f32, tag=f"rb{s}", name=f"rb{s}")
                    nc.sync.dma_start(rb[:], bass.AP(
                        tensor=bc_d.ap().tensor, offset=(2*s+1)*TQ,
                        ap=[[0, 128], [1, TQ]]))
                    stats.append((mb, rb))

        # =================== LN1 normalize + FFN l1 ======================
        with tc.tile_pool(name="ynp", bufs=1) as ynp:
            yn_sb = [ynp.tile([128, KC_D, TQ], f32r, tag=f"yn{v}", name=f"yn{v}")
                     for v in range(3)]           # ynr', yni'', neg-yni''
            with tc.tile_pool(name="ln_t", bufs=3) as tp:
                for kc, (r0, rsz) in enumerate(_chunks(D)):
                    for s in range(2):
                        mb, rb = stats[s]
                        yl = tp.tile([128, TQ], f32, tag=f"yl{s}")
                        nc.sync.dma_start(yl[:rsz, :], y_d[s].ap()[r0:r0+rsz, :])
                        t_ = tp.tile([128, TQ], f32, tag=f"n{s}")
                        nc.vector.tensor_sub(t_[:rsz, :], yl[:rsz, :],
                                             mb[:rsz, :])
                        nc.vector.tensor_mul(t_[:rsz, :], t_[:rsz, :],
                                             rb[:rsz, :])
                        if s == 0:
                            nc.vector.tensor_scalar(
                                out=yn_sb[0][:rsz, kc, :], in0=t_[:rsz, :],
                                scalar1=g1rsb[:rsz, kc:kc+1], scalar2=None,
                                op0=ALU.mult)
                        else:
                            nc.vector.tensor_scalar(
                                out=yn_sb[1][:rsz, kc, :], in0=t_[:rsz, :],
                                scalar1=g1isb[:rsz, kc:kc+1], scalar2=None,
                                op0=ALU.mult)
                            nc.vector.tensor_scalar(
                                out=yn_sb[2][:rsz, kc, :], in0=t_[:rsz, :],
                                scalar1=g1isb[:rsz, kc:kc+1], scalar2=-1.0,
                                op0=ALU.mult, op1=ALU.mult)
                lastc = (D + 1 - 1) // 128
                lastp = D - lastc * 128
                nc.sync.dma_start(yn_sb[0][lastp:lastp+1, lastc, :],
                                  ones_in.ap()[:, :TQ])
                nc.sync.dma_start(yn_sb[1][lastp:lastp+1, lastc, :],
                                  zeros_in.ap()[:, :TQ])
                nc.sync.dma_start(yn_sb[2][lastp:lastp+1, lastc, :],
                                  zeros_in.ap()[:, :TQ])

            # FFN l1 with batched (pre-tiled) weight loads
            with tc.tile_pool(name="l1_ps", bufs=4, space="PSUM") as psp, \
                 tc.tile_pool(name="l1_w", bufs=2) as wp, \
                 tc.tile_pool(name="l1_o", bufs=3) as op_:
                for mci, (mc, msz) in enumerate(_chunks(DFF)):
                    wtr = wp.tile([128, KC_D, 128], f32r, tag="wtr")
                    nc.sync.dma_start(wtr[:], w1r_t.ap()[:, mci])
                    wti = wp.tile([128, KC_D, 128], f32r, tag="wti")
                    nc.sync.dma_start(wti[:], w1i_t.ap()[:, mci])
                    phr = psp.tile([128, TQ], f32, tag="phr")
                    phi = psp.tile([128, TQ], f32, tag="phi")
                    for kc, (r0, rsz) in enumerate(_chunks(D + 1)):
                        nc.tensor.matmul(phr[:msz, :], wtr[:rsz, kc, :msz],
                                         yn_sb[0][:rsz, kc, :],
                                         start=(kc == 0), stop=False)
                        nc.tensor.matmul(phr[:msz, :], wti[:rsz, kc, :msz],
                                         yn_sb[2][:rsz, kc, :],
                                         start=False, stop=(kc == KC_D - 1))
                        nc.tensor.matmul(phi[:msz, :], wti[:rsz, kc, :msz],
                                         yn_sb[0][:rsz, kc, :],
                                         start=(kc == 0), stop=False)
                        nc.tensor.matmul(phi[:msz, :], wtr[:rsz, kc, :msz],
                                         yn_sb[1][:rsz, kc, :],
                                         start=False, stop=(kc == KC_D - 1))
                    hro = op_.tile([128, TQ], f32r, tag="hro")
                    nc.scalar.activation(out=hro[:msz, :], in_=phr[:msz, :],
                                         func=AF.Relu)
                    nc.sync.dma_start(h_d[0].ap()[mc:mc+msz, :], hro[:msz, :])
                    hio = op_.tile([128, TQ], f32r, tag="hio")
                    nc.scalar.activation(out=hio[:msz, :], in_=phi[:msz, :],
                                         func=AF.Relu)
                    nc.sync.dma_start(h_d[1].ap()[mc:mc+msz, :], hio[:msz, :])
                    hin = op_.tile([128, TQ], f32r, tag="hin")
                    nc.vector.tensor_scalar(
                        out=hin[:msz, :], in0=phi[:msz, :], scalar1=-1.0,
                        scalar2=0.0, op0=ALU.mult, op1=ALU.min)
                    nc.sync.dma_start(h_d[2].ap()[mc:mc+msz, :], hin[:msz, :])

        # =================== FFN l2 (kc-blocked, zT accum in SBUF) =======
        with tc.tile_pool(name="l2_acc", bufs=1) as accp:
            z_sb = [accp.tile([128, KC_D, TQ], f32, tag=f"z{s}", name=f"zacc{s}") for s in range(2)]
            with tc.tile_pool(name="l2_h", bufs=2) as hp, \
                 tc.tile_pool(name="l2_ps", bufs=2, space="PSUM") as psp, \
                 tc.tile_pool(name="l2_w", bufs=6) as wp:
                nblk = (KC_H + L2_BLK - 1) // L2_BLK
                for blk in range(nblk):
                    k0 = blk * L2_BLK
                    kn = min(L2_BLK, KC_H - k0)
                    hb = [hp.tile([128, L2_BLK, TQ], f32r, tag=f"hb{v}", name=f"hb{v}")
                          for v in range(3)]
                    for v in range(3):
                        for kk in range(kn):
                            r0 = (k0 + kk) * 128
                            rsz = min(128, DFF + 1 - r0)
                            nc.sync.dma_start(hb[v][:rsz, kk, :],
                                              h_d[v].ap()[r0:r0+rsz, :])
                    for mc, msz in _chunks(D):
                        mci = mc // 128
                        pzr = psp.tile([128, TQ], f32, tag="pzr")
                        pzi = psp.tile([128, TQ], f32, tag="pzi")
                        mci = mc // 128
                        wtr = wp.tile([128, L2_BLK, 128], f32r, tag="wtr")
                        nc.sync.dma_start(wtr[:, :kn, :],
                                          w2r_t.ap()[:, mci, k0:k0+kn, :])
                        wti = wp.tile([128, L2_BLK, 128], f32r, tag="wti")
                        nc.sync.dma_start(wti[:, :kn, :],
                                          w2i_t.ap()[:, mci, k0:k0+kn, :])
                        for kk in range(kn):
                            r0 = (k0 + kk) * 128
                            rsz = min(128, DFF + 1 - r0)
                            nc.tensor.matmul(pzr[:msz, :], wtr[:rsz, kk, :msz],
                                             hb[0][:rsz, kk, :],
                                             start=(kk == 0), stop=False)
                            nc.tensor.matmul(pzr[:msz, :], wti[:rsz, kk, :msz],
                                             hb[2][:rsz, kk, :],
                                             start=False, stop=(kk == kn - 1))
                            nc.tensor.matmul(pzi[:msz, :], wti[:rsz, kk, :msz],
                                             hb[0][:rsz, kk, :],
                                             start=(kk == 0), stop=False)
                            nc.tensor.matmul(pzi[:msz, :], wtr[:rsz, kk, :msz],
                                             hb[1][:rsz, kk, :],
                                             start=False, stop=(kk == kn - 1))
                        if blk == 0:
                            nc.scalar.copy(z_sb[0][:msz, mci, :], pzr[:msz, :])
                            nc.scalar.copy(z_sb[1][:msz, mci, :], pzi[:msz, :])
                        else:
                            nc.vector.tensor_add(z_sb[0][:msz, mci, :],
                                                 z_sb[0][:msz, mci, :],
                                                 pzr[:msz, :])
                            nc.vector.tensor_add(z_sb[1][:msz, mci, :],
                                                 z_sb[1][:msz, mci, :],
                                                 pzi[:msz, :])
            if debug:
                for s in range(2):
                    for kc, (r0, rsz) in enumerate(_chunks(D)):
                        nc.sync.dma_start(z_tap[s].ap()[r0:r0+rsz, :],
                                          z_sb[s][:rsz, kc, :])

            # =================== LN2 + mask + spec2 ======================
            sp2m = [nc.dram_tensor(f"sp2m_{s}", (D, TQ), f32r, kind="Internal")
                    for s in range(2)]
            with tc.tile_pool(name="ln2_ps", bufs=1, space="PSUM") as psp, \
                 tc.tile_pool(name="ln2_t", bufs=2) as tp:
                stats2 = []
                for s in range(2):
                    pm = psp.tile([1, TQ], f32, tag=f"pm{s}")
                    pv = psp.tile([1, TQ], f32, tag=f"pv{s}")
                    for kc, (r0, rsz) in enumerate(_chunks(D)):
                        nc.tensor.matmul(pm[:1, :], ones_col[:rsz, :],
                                         z_sb[s][:rsz, kc, :],
                                         start=(kc == 0), stop=(kc == MC_D_LAST))
                        sq = tp.tile([128, TQ], f32, tag="sq")
                        nc.scalar.activation(out=sq[:rsz, :],
                                             in_=z_sb[s][:rsz, kc, :], func=AF.Square)
                        nc.tensor.matmul(pv[:1, :], ones_col[:rsz, :], sq[:rsz, :],
                                         start=(kc == 0), stop=(kc == MC_D_LAST))
                    mT = tp.tile([1, TQ], f32, tag=f"m{s}")
                    nc.scalar.mul(mT[:1, :], pm[:1, :], 1.0 / D)
                    msq = tp.tile([1, TQ], f32, tag=f"msq{s}")
                    nc.scalar.activation(out=msq[:1, :], in_=mT[:1, :], func=AF.Square)
                    var = tp.tile([1, TQ], f32, tag=f"var{s}")
                    nc.scalar.mul(var[:1, :], pv[:1, :], 1.0 / D)
                    nc.vector.tensor_sub(var[:1, :], var[:1, :], msq[:1, :])
                    nc.scalar.activation(out=var[:1, :], in_=var[:1, :],
                                         func=AF.Sqrt, bias=eps_t[:1, :])
                    rs = tp.tile([1, TQ], f32, tag=f"rs{s}")
                    nc.vector.reciprocal(rs[:1, :], var[:1, :])
                    nc.sync.dma_start(bc_d.ap()[4+2*s:5+2*s, :], mT[:1, :])
                    nc.sync.dma_start(bc_d.ap()[5+2*s:6+2*s, :], rs[:1, :])
                    mb = tp.tile([128, TQ], f32, tag=f"mb{s}")
                    nc.sync.dma_start(mb[:], bass.AP(
                        tensor=bc_d.ap().tensor, offset=(4+2*s)*TQ,
                        ap=[[0, 128], [1, TQ]]))
                    rb = tp.tile([128, TQ], f32, tag=f"rb{s}")
                    nc.sync.dma_start(rb[:], bass.AP(
                        tensor=bc_d.ap().tensor, offset=(5+2*s)*TQ,
                        ap=[[0, 128], [1, TQ]]))
                    stats2.append((mb, rb))
                for kc, (r0, rsz) in enumerate(_chunks(D)):
                    zn = []
                    for s in range(2):
                        mb, rb = stats2[s]
                        t_ = tp.tile([128, TQ], f32, tag=f"zn{s}")
                        nc.vector.tensor_sub(t_[:rsz, :], z_sb[s][:rsz, kc, :],
                                             mb[:rsz, :])
                        nc.vector.tensor_mul(t_[:rsz, :], t_[:rsz, :], rb[:rsz, :])
                        zn.append(t_)
                    sqr = tp.tile([128, TQ], f32, tag="sqr")
                    nc.sync.dma_start(sqr[:rsz, :], specq_d[0].ap()[r0:r0+rsz, :])
                    sqi = tp.tile([128, TQ], f32, tag="sqi")
                    nc.sync.dma_start(sqi[:rsz, :], specq_d[1].ap()[r0:r0+rsz, :])
                    # m_r = zn_r*Ar - zn_i*Br + Cr ; m_i = zn_i*Ai + zn_r*Bi + Ci
                    t1 = tp.tile([128, TQ], f32, tag="mk1")
                    nc.vector.tensor_scalar(
                        out=t1[:rsz, :], in0=zn[0][:rsz, :],
                        scalar1=dvsb[:rsz, kc, 0:1], scalar2=dvsb[:rsz, kc, 2:3],
                        op0=ALU.mult, op1=ALU.add)
                    t2 = tp.tile([128, TQ], f32, tag="mk2")
                    nc.vector.tensor_scalar(
                        out=t2[:rsz, :], in0=zn[1][:rsz, :],
                        scalar1=dvsb[:rsz, kc, 1:2], scalar2=None, op0=ALU.mult)
                    nc.vector.tensor_sub(t1[:rsz, :], t1[:rsz, :], t2[:rsz, :])
                    nc.scalar.activation(out=t1[:rsz, :], in_=t1[:rsz, :],
                                         func=AF.Sigmoid)
                    o_r = tp.tile([128, TQ], f32, tag="o_r")
                    nc.vector.tensor_mul(o_r[:rsz, :], sqr[:rsz, :], t1[:rsz, :])
                    nc.sync.dma_start(spec2_r.ap()[r0:r0+rsz, :], o_r[:rsz, :])
                    o_rm = tp.tile([128, TQ], f32r, tag="o_rm")
                    nc.vector.tensor_mul(o_rm[:rsz, :], o_r[:rsz, :],
                                         cmask_b[:rsz, :])
                    nc.sync.dma_start(sp2m[0].ap()[r0:r0+rsz, :], o_rm[:rsz, :])
                    t3 = tp.tile([128, TQ], f32, tag="mk3")
                    nc.vector.tensor_scalar(
                        out=t3[:rsz, :], in0=zn[1][:rsz, :],
                        scalar1=dvsb[:rsz, kc, 3:4], scalar2=dvsb[:rsz, kc, 5:6],
                        op0=ALU.mult, op1=ALU.add)
                    t4 = tp.tile([128, TQ], f32, tag="mk4")
                    nc.vector.tensor_scalar(
                        out=t4[:rsz, :], in0=zn[0][:rsz, :],
                        scalar1=dvsb[:rsz, kc, 4:5], scalar2=None, op0=ALU.mult)
                    nc.vector.tensor_add(t3[:rsz, :], t3[:rsz, :], t4[:rsz, :])
                    nc.scalar.activation(out=t3[:rsz, :], in_=t3[:rsz, :],
                                         func=AF.Sigmoid)
                    o_i = tp.tile([128, TQ], f32, tag="o_i")
                    nc.vector.tensor_mul(o_i[:rsz, :], sqi[:rsz, :], t3[:rsz, :])
                    nc.sync.dma_start(spec2_i.ap()[r0:r0+rsz, :], o_i[:rsz, :])
                    o_im = tp.tile([128, TQ], f32r, tag="o_im")
                    nc.vector.tensor_mul(o_im[:rsz, :], o_i[:rsz, :],
                                         cmask_b[:rsz, :])
                    nc.sync.dma_start(sp2m[1].ap()[r0:r0+rsz, :], o_im[:rsz, :])

        # =================== iSTFT + overlap-add =====================
        with tc.tile_pool(name="ist", bufs=1) as istp, \
             tc.tile_pool(name="ist_ps", bufs=4, space="PSUM") as psp, \
             tc.tile_pool(name="ist_w", bufs=6) as wp, \
             tc.tile_pool(name="ist_t", bufs=4) as tp:
            for ch in range(2):
                # load rhs: masked spec rows for this channel, bin-chunked
                rsp = []
                for s in range(2):
                    r_ = istp.tile([128, KC_HD, TQ], f32r, tag=f"rsp{s}")
                    for kc, (r0, rsz) in enumerate(_chunks(BINS)):
                        nc.sync.dma_start(
                            r_[:rsz, kc, :],
                            sp2m[s].ap()[ch*BINS+r0 : ch*BINS+r0+rsz, :])
                    rsp.append(r_)
                bsum = istp.tile([128, 4, SEG_S], f32, tag="bsum")
                nc.vector.memset(bsum[:], 0.0)
                for mn in range(KC_N):
                    pf = psp.tile([128, TQ], f32, tag="pf")
                    ct = wp.tile([128, KC_HD, 128], f32r, tag="ct")
                    nc.sync.dma_start(ct[:], istft_c.ap()[:, mn])
                    st = wp.tile([128, KC_HD, 128], f32r, tag="st")
                    nc.sync.dma_start(st[:], istft_s.ap()[:, mn])
                    for kc, (r0, rsz) in enumerate(_chunks(BINS)):
                        nc.tensor.matmul(pf[:, :], ct[:rsz, kc, :],
                                         rsp[0][:rsz, kc, :],
                                         start=(kc == 0), stop=False)
                        nc.tensor.matmul(pf[:, :], st[:rsz, kc, :],
                                         rsp[1][:rsz, kc, :],
                                         start=False, stop=(kc == KC_HD - 1))
                    j = mn // 4
                    mcb = mn % 4
                    nc.vector.tensor_add(bsum[:, mcb, j:j+TQ], bsum[:, mcb, j:j+TQ],
                                         pf[:, :])
                # transpose bsum -> [s, 512] and write out
                for sc, ssz in _chunks(SEG_S):
                    for mcb in range(4):
                        pst = psp.tile([128, 128], f32, tag="pst")
                        nc.tensor.transpose(pst[:ssz, :], bsum[:, mcb, sc:sc+ssz],
                                            ident[:, :])
                        so = tp.tile([128, 128], f32, tag="so")
                        nc.scalar.copy(so[:ssz, :], pst[:ssz, :])
                        dst = bass.AP(
                            tensor=seg_o.ap().tensor,
                            offset=ch * SEG + sc * 512 + mcb * 128,
                            ap=[[512, ssz], [1, 128]])
                        nc.sync.dma_start(dst, so[:ssz, :])

    nc.compile()
    return nc


MC_D_LAST = 16  # last chunk index of D rows (17 chunks)


# ---------------------------------------------------------------------------
# kernel entry
# ---------------------------------------------------------------------------

_CACHE = {}


def _get_program(debug=False):
    key = ("prog", debug)
    if key not in _CACHE:
        _CACHE[key] = _build_program(debug=debug)
    return _CACHE[key]


def _install_neff_cache():
    """Disk-cache walrus NEFF compiles keyed by BIR hash (compiles are
    10+ minutes; identical BIR -> identical NEFF)."""
    import hashlib
    import shutil
    from concourse import bass_utils, bass2jax
    if getattr(bass_utils, '_neff_cache_installed', False):
        return
    orig = bass_utils.compile_bir_kernel

    def cached(bir_json, tmpdir, neff_name="file.neff"):
        h = hashlib.sha256(bir_json).hexdigest()[:24]
        cdir = os.environ.get("BASS_NEFF_CACHE", "/tmp/bass_neff_cache")
        os.makedirs(cdir, exist_ok=True)
        cpath = os.path.join(cdir, f"{h}_{neff_name}")
        if os.path.exists(cpath):
            dst = os.path.join(tmpdir, neff_name)
            shutil.copy(cpath, dst)
            return dst
        p = orig(bir_json, tmpdir, neff_name=neff_name)
        try:
            shutil.copy(p, cpath)
        except OSError:
            pass
        return p

    bass_utils.compile_bir_kernel = cached
    bass2jax.compile_bir_kernel = cached
    bass_utils._neff_cache_installed = True


def kernel(debug=False, _run_kwargs=None, **inputs):
    from concourse import bass_utils
    _install_neff_cache()

    consts = _prep_constants(inputs)
    wsq = _prep_wsq(inputs['window'])
    mix = np.asarray(inputs['mix'], np.float32)

    pe_own = [np.ascontiguousarray(consts['peT'][:, o:o+TQ]) for o in Q_OFF]
    # half1 overlaps half0 by 2 frames; zero them out of its iSTFT input
    cmask = [np.ones((1, TQ), np.float32), np.ones((1, TQ), np.float32)]
    cmask[1][:, :2] = 0.0

    shared = {k: consts[k] for k in
              ('stft_c', 'stft_s', 'istft_c', 'istft_s', 'c1sc', 'dvec',
               'g1r', 'g1i', 'wq0', 'wq1', 'wk0', 'wk1', 'wv0', 'wv1',
               'wo', 'w1rp', 'w1ip', 'w2rp', 'w2ip')}
    shared['pe_full'] = consts['peT']
    shared['ones_in'] = np.ones((1, T), np.float32)
    shared['zeros_in'] = np.zeros((1, T), np.float32)
    shared['ones32_in'] = np.ones((1, T), np.float32)

    in_maps = []
    for core in range(8):
        b, half = core // 2, core % 2
        apad = np.pad(mix[b], ((0, 0), (PAD, PAD)), mode='reflect')
        m = dict(shared)
        m['audio_full'] = np.ascontiguousarray(apad)
        o = Q_OFF[half] * HOP
        m['audio_own'] = np.ascontiguousarray(apad[:, o:o + SEG])
        m['pe_own'] = pe_own[half]
        m['colmask'] = cmask[half]
        in_maps.append(m)

    prog = _get_program(debug=debug)
    rk = _run_kwargs or {}
    res = bass_utils.run_bass_kernel_spmd(prog, in_maps, core_ids=list(range(8)),
                                          **rk)

    # gather
    p = consts['perm']
    inv = np.empty(D, np.int64); inv[p] = np.arange(D)
    est = np.zeros((B, 2, L), np.float32)
    spec_stack = np.zeros((B * 2, BINS, T, 2), np.float32)
    for b in range(B):
        ola = np.zeros((2, APLEN), np.float64)
        for half in range(2):
            r = res.results[b * 2 + half]
            ola[:, Q_OFF[half]*HOP : Q_OFF[half]*HOP + SEG] += r['seg']
            c0 = 0 if half == 0 else 2
            t0 = Q_OFF[half] + c0
            for ch in range(2):
                spec_stack[b*2+ch, :, t0:Q_OFF[half]+TQ, 0] = \
                    r['spec2_r'][ch*BINS:(ch+1)*BINS, c0:]
                spec_stack[b*2+ch, :, t0:Q_OFF[half]+TQ, 1] = \
                    r['spec2_i'][ch*BINS:(ch+1)*BINS, c0:]
        ola = ola / wsq[None, :]
        est[b] = ola[:, PAD:PAD+L].astype(np.float32)
    if debug:
        kernel.last_results = res
    kernel.last_exec_time_ns = getattr(res, 'exec_time_ns', None)
    return est, spec_stack
